# revision 31
# baseline (speedup 1.0000x reference)
import sys, time
sys.path.insert(0, "/opt/trn_rl_repo")
import numpy as np
from concourse import bass, bacc, mybir, tile
from concourse.bass_utils import run_bass_kernel_spmd

# Problem constants (nn_Memory_88656714925588)
B, CK, CV = 1, 64, 256
H, W, T = 64, 64, 8
NE = H * W * T            # 32768 memory elements
Q = H * W * 64 // 64      # 4096 queries
NC = 8                    # cores
Q_LOC = Q // NC           # 512 queries per core (query-sharded)
NQT = Q_LOC // 128        # 4 query tiles per core
TOPK = 20
CW = 64                   # chunk width for the screen
NCH = NE // CW            # 512 chunks per query row
NSEL = 24                 # chunks selected per query (>= 20 guarantees coverage)
NCAND = NSEL * 8          # 192 candidates after per-chunk top-8
NSLICE = 8                # 4096-column slices per tile
SLW = NE // NSLICE        # 4096
F32 = mybir.dt.float32
F16 = mybir.dt.float16
U32 = mybir.dt.uint32
NEG = -1e30
EPS = 2.0 ** -21

_prog_cache = {}


def _build_program():
    if "p" in _prog_cache:
        return _prog_cache["p"]
    nc = bacc.Bacc()
    qTb = nc.dram_tensor("qTb", [CK + 2, Q_LOC], F16, kind="ExternalInput")
    mkB = nc.dram_tensor("mkB", [CK + 2, NE], F16, kind="ExternalInput")
    eps1536 = nc.dram_tensor(
        "eps1536", [128, NSEL * CW], F32, kind="ExternalInput")
    vTb = nc.dram_tensor("vTb", [NE, 2 * CV], F16, kind="ExternalInput")
    mkT32 = nc.dram_tensor("mkT32", [NE, CK + 1], F32, kind="ExternalInput")
    qrI = nc.dram_tensor("qrI", [Q_LOC, CK + 1], F32, kind="ExternalInput")
    prow512 = nc.dram_tensor("prow512", [128, 1], F32, kind="ExternalInput")
    prow192 = nc.dram_tensor("prow192", [128, 1], F32, kind="ExternalInput")
    prow24 = nc.dram_tensor("prow24", [128, 1], F32, kind="ExternalInput")
    eps512 = nc.dram_tensor("eps512", [128, NCH], F32, kind="ExternalInput")
    eps192 = nc.dram_tensor("eps192", [128, NCAND], F32, kind="ExternalInput")
    out = nc.dram_tensor("out", [Q_LOC, 2 * CV], F32, kind="ExternalOutput")

    with tile.TileContext(nc) as tc:
        with tc.tile_pool(name="cst", bufs=1) as cst, \
             tc.tile_pool(name="aff", bufs=3) as affp, \
             tc.tile_pool(name="tree", bufs=2) as tre, \
             tc.tile_pool(name="sel", bufs=2) as sel, \
             tc.tile_pool(name="gat", bufs=2) as gat, \
             tc.tile_pool(name="psum", bufs=2, space="PSUM") as psum, \
             tc.tile_pool(name="dram", bufs=2, space="DRAM") as dram:

            qt = cst.tile([CK + 2, Q_LOC], F16)
            mkt = cst.tile([CK + 2, NE], F16)
            # chunked mk load so the first matmuls start early
            for ci in range(NSLICE):
                nc.sync.dma_start(
                    out=mkt[:, ci * SLW:(ci + 1) * SLW],
                    in_=mkB[:, ci * SLW:(ci + 1) * SLW])
            nc.sync.dma_start(out=qt[:], in_=qTb[:])
            ep1536 = cst.tile([128, NSEL * CW], F32)
            nc.sync.dma_start(out=ep1536[:], in_=eps1536[:])
            pr512 = cst.tile([128, 1], F32)
            nc.sync.dma_start(out=pr512[:], in_=prow512[:])
            pr192 = cst.tile([128, 1], F32)
            nc.sync.dma_start(out=pr192[:], in_=prow192[:])
            pr24 = cst.tile([128, 1], F32)
            nc.sync.dma_start(out=pr24[:], in_=prow24[:])
            ep512 = cst.tile([128, NCH], F32)
            nc.sync.dma_start(out=ep512[:], in_=eps512[:])
            ep192 = cst.tile([128, NCAND], F32)
            nc.sync.dma_start(out=ep192[:], in_=eps192[:])

            for t in range(NQT):
                qrt = sel.tile([128, CK + 1], F32, tag="qrt")
                nc.sync.dma_start(
                    out=qrt[:], in_=qrI[t * 128:(t + 1) * 128, :])
                affsD = dram.tile([128 * NCH, CW], F16, tag="affsD")
                affsDv = affsD[:].rearrange("(p c) w -> p (c w)", p=128)
                elD = dram.tile([128 * NCAND, 1], F32, tag="elD")
                cmax = tre.tile([128, NCH], F16, tag="cmax")

                for s in range(NSLICE):
                    aff4 = affp.tile([128, SLW], F16, tag="aff4")
                    for h in range(2):
                        ph = psum.tile([128, 2048], F32, tag="ph")
                        for c in range(4):
                            col = s * SLW + h * 2048 + c * 512
                            nc.tensor.matmul(
                                out=ph[:, c * 512:(c + 1) * 512],
                                lhsT=qt[:, t * 128:(t + 1) * 128],
                                rhs=mkt[:, col:col + 512],
                                start=True, stop=True)
                        nc.scalar.activation(
                            out=aff4[:, h * 2048:(h + 1) * 2048], in_=ph[:],
                            func=mybir.ActivationFunctionType.Copy)
                    # stage this slice to DRAM for the per-query rescan gathers
                    eng = nc.sync if s % 2 == 0 else nc.gpsimd
                    eng.dma_start(
                        out=affsDv[:, s * SLW:(s + 1) * SLW], in_=aff4[:])
                    # chunk-local pairwise-max tree: 4096 -> 64 chunk maxima
                    a3 = aff4[:].rearrange("p (g w) -> p g w", w=CW)
                    t1 = tre.tile([128, 2048], F16, tag="t1")
                    nc.vector.tensor_tensor(
                        out=t1[:].rearrange("p (g w) -> p g w", w=32),
                        in0=a3[:, :, 0:32], in1=a3[:, :, 32:64],
                        op=mybir.AluOpType.max)
                    t2 = tre.tile([128, 1024], F16, tag="t2")
                    nc.vector.tensor_tensor(
                        out=t2[:].rearrange("p (g w) -> p g w", w=16),
                        in0=t1[:].rearrange("p (g w) -> p g w", w=32)[:, :, 0:16],
                        in1=t1[:].rearrange("p (g w) -> p g w", w=32)[:, :, 16:32],
                        op=mybir.AluOpType.max)
                    t3 = tre.tile([128, 512], F16, tag="t3")
                    nc.vector.tensor_tensor(
                        out=t3[:].rearrange("p (g w) -> p g w", w=8),
                        in0=t2[:].rearrange("p (g w) -> p g w", w=16)[:, :, 0:8],
                        in1=t2[:].rearrange("p (g w) -> p g w", w=16)[:, :, 8:16],
                        op=mybir.AluOpType.max)
                    t4 = tre.tile([128, 256], F16, tag="t4")
                    nc.vector.tensor_tensor(
                        out=t4[:].rearrange("p (g w) -> p g w", w=4),
                        in0=t3[:].rearrange("p (g w) -> p g w", w=8)[:, :, 0:4],
                        in1=t3[:].rearrange("p (g w) -> p g w", w=8)[:, :, 4:8],
                        op=mybir.AluOpType.max)
                    t5 = tre.tile([128, 128], F16, tag="t5")
                    nc.vector.tensor_tensor(
                        out=t5[:].rearrange("p (g w) -> p g w", w=2),
                        in0=t4[:].rearrange("p (g w) -> p g w", w=4)[:, :, 0:2],
                        in1=t4[:].rearrange("p (g w) -> p g w", w=4)[:, :, 2:4],
                        op=mybir.AluOpType.max)
                    nc.vector.tensor_tensor(
                        out=cmax[:, s * 64:(s + 1) * 64],
                        in0=t5[:].rearrange("p (g w) -> p g w", w=2)[:, :, 0],
                        in1=t5[:].rearrange("p (g w) -> p g w", w=2)[:, :, 1],
                        op=mybir.AluOpType.max)

                # ---- select top-NSEL chunks per query (tie-free in f32) ----
                cmaxf = sel.tile([128, NCH], F32, tag="cmaxf")
                nc.vector.tensor_copy(cmaxf[:], cmax[:])
                nc.vector.tensor_tensor(
                    out=cmaxf[:], in0=cmaxf[:], in1=ep512[:],
                    op=mybir.AluOpType.add)
                cidu = sel.tile([128, NSEL], U32, tag="cidu")
                m8 = sel.tile([128, 8], F32, tag="m8")
                for r in range(NSEL // 8):
                    nc.vector.max(out=m8[:], in_=cmaxf[:])
                    nc.vector.max_index(
                        out=cidu[:, r * 8:(r + 1) * 8], in_max=m8[:],
                        in_values=cmaxf[:])
                    if r < NSEL // 8 - 1:
                        nc.vector.match_replace(
                            out=cmaxf[:], in_to_replace=m8[:],
                            in_values=cmaxf[:], imm_value=NEG)
                cidf = sel.tile([128, NSEL], F32, tag="cidf")
                nc.vector.tensor_copy(cidf[:], cidu[:])
                offf = sel.tile([128, NSEL], F32, tag="offf")
                nc.vector.tensor_scalar(
                    offf[:], cidf[:], pr512[:], None, op0=mybir.AluOpType.add)
                offu = sel.tile([128, NSEL], U32, tag="offu")
                nc.vector.tensor_copy(offu[:], offf[:])

                # ---- gather the selected chunks, rescan for top-8 each ----
                g24 = gat.tile([128, NSEL * CW], F16, tag="g24")
                nc.gpsimd.indirect_dma_start(
                    out=g24[:].rearrange("p (k w) -> p k w", w=CW),
                    out_offset=None, in_=affsD[:],
                    in_offset=bass.IndirectOffsetOnAxis(ap=offu[:], axis=0))
                # f32 + positional eps makes the rescan tie-free, so
                # max_index can't alias two tied elements to one position
                g24f = gat.tile([128, NSEL * CW], F32, tag="g24f")
                nc.vector.scalar_tensor_tensor(
                    out=g24f[:], in0=g24[:], scalar=1.0, in1=ep1536[:],
                    op0=mybir.AluOpType.mult, op1=mybir.AluOpType.add)
                cv8 = sel.tile([128, NCAND], F32, tag="cv8")
                pix = sel.tile([128, NCAND], U32, tag="pix")
                for j in range(NSEL):
                    nc.vector.max(
                        out=cv8[:, j * 8:(j + 1) * 8],
                        in_=g24f[:, j * CW:(j + 1) * CW])
                    nc.vector.max_index(
                        out=pix[:, j * 8:(j + 1) * 8],
                        in_max=cv8[:, j * 8:(j + 1) * 8],
                        in_values=g24f[:, j * CW:(j + 1) * CW])
                # decode element index: el = cid*64 + pix
                pixf = sel.tile([128, NCAND], F32, tag="pixf")
                nc.vector.tensor_copy(pixf[:], pix[:])
                elf = sel.tile([128, NCAND], F32, tag="elf")
                nc.vector.scalar_tensor_tensor(
                    out=elf[:].rearrange("p (k r) -> p k r", r=8),
                    in0=cidf[:].rearrange("p (k u) -> p k u", u=1)
                    .broadcast_to([128, NSEL, 8]),
                    scalar=float(CW),
                    in1=pixf[:].rearrange("p (k r) -> p k r", r=8),
                    op0=mybir.AluOpType.mult, op1=mybir.AluOpType.add)
                nc.sync.dma_start(
                    out=elD[:].rearrange("(p u) one -> p (u one)", p=128),
                    in_=elf[:])

                # ---- merge: exact top-20 of the 192 candidates ----
                cvf = sel.tile([128, NCAND], F32, tag="cvf")
                nc.vector.tensor_tensor(
                    out=cvf[:], in0=cv8[:], in1=ep192[:],
                    op=mybir.AluOpType.add)
                gvals = sel.tile([128, 24], F32, tag="gvals")
                gpos = sel.tile([128, 24], U32, tag="gpos")
                for r in range(3):
                    g8 = gvals[:, r * 8:(r + 1) * 8]
                    nc.vector.max(out=g8, in_=cvf[:])
                    nc.vector.max_index(
                        out=gpos[:, r * 8:(r + 1) * 8], in_max=g8,
                        in_values=cvf[:])
                    if r < 2:
                        nc.vector.match_replace(
                            out=cvf[:], in_to_replace=g8, in_values=cvf[:],
                            imm_value=NEG)
                # ---- exact fp32 rescore of the 24 candidates ----
                gposf = sel.tile([128, 24], F32, tag="gposf")
                nc.vector.tensor_copy(gposf[:], gpos[:])
                off2 = sel.tile([128, 24], F32, tag="off2")
                nc.vector.tensor_scalar(
                    off2[:], gposf[:], pr192[:], None, op0=mybir.AluOpType.add)
                offu2 = sel.tile([128, 24], U32, tag="offu2")
                nc.vector.tensor_copy(offu2[:], off2[:])
                el24 = sel.tile([128, 24], F32, tag="el24")
                nc.gpsimd.indirect_dma_start(
                    out=el24[:], out_offset=None, in_=elD[:],
                    in_offset=bass.IndirectOffsetOnAxis(ap=offu2[:], axis=0))
                el24u = sel.tile([128, 24], U32, tag="el24u")
                nc.vector.tensor_copy(el24u[:], el24[:])
                gmk = gat.tile([128, 24 * (CK + 1)], F32, tag="gmk")
                nc.gpsimd.indirect_dma_start(
                    out=gmk[:].rearrange("p (k c) -> p k c", c=CK + 1),
                    out_offset=None, in_=mkT32[:],
                    in_offset=bass.IndirectOffsetOnAxis(ap=el24u[:], axis=0))
                nc.vector.tensor_tensor(
                    out=gmk[:].rearrange("p (k c) -> p k c", c=CK + 1),
                    in0=gmk[:].rearrange("p (k c) -> p k c", c=CK + 1),
                    in1=qrt[:].rearrange("p (u c) -> p u c", u=1)
                    .broadcast_to([128, 24, CK + 1]),
                    op=mybir.AluOpType.mult)
                av24 = sel.tile([128, 24], F32, tag="av24")
                nc.vector.tensor_reduce(
                    out=av24[:],
                    in_=gmk[:].rearrange("p (k c) -> p k c", c=CK + 1),
                    axis=mybir.AxisListType.X, op=mybir.AluOpType.add)
                nc.vector.tensor_tensor(
                    out=av24[:], in0=av24[:], in1=ep192[:, :24],
                    op=mybir.AluOpType.add)
                # exact top-20 of the rescored 24
                wvals = sel.tile([128, 24], F32, tag="wvals")
                wpos = sel.tile([128, 24], U32, tag="wpos")
                for r in range(3):
                    w8 = wvals[:, r * 8:(r + 1) * 8]
                    nc.vector.max(out=w8, in_=av24[:])
                    nc.vector.max_index(
                        out=wpos[:, r * 8:(r + 1) * 8], in_max=w8,
                        in_values=av24[:])
                    if r < 2:
                        nc.vector.match_replace(
                            out=av24[:], in_to_replace=w8, in_values=av24[:],
                            imm_value=NEG)
                # winner element ids via a staged lookup of el24
                el24D = dram.tile([128 * 24, 1], F32, tag="el24D")
                nc.sync.dma_start(
                    out=el24D[:].rearrange("(p u) one -> p (u one)", p=128),
                    in_=el24[:])
                wposf = sel.tile([128, TOPK], F32, tag="wposf")
                nc.vector.tensor_copy(wposf[:], wpos[:, :TOPK])
                offw = sel.tile([128, TOPK], F32, tag="offw")
                nc.vector.tensor_scalar(
                    offw[:], wposf[:], pr24[:], None, op0=mybir.AluOpType.add)
                offwu = sel.tile([128, TOPK], U32, tag="offwu")
                nc.vector.tensor_copy(offwu[:], offw[:])
                elw = sel.tile([128, TOPK], F32, tag="elw")
                nc.gpsimd.indirect_dma_start(
                    out=elw[:], out_offset=None, in_=el24D[:],
                    in_offset=bass.IndirectOffsetOnAxis(ap=offwu[:], axis=0))
                iku = sel.tile([128, TOPK], U32, tag="iku")
                nc.vector.tensor_copy(iku[:], elw[:])

                # ---- softmax over the exact top-20 values ----
                negm = sel.tile([128, 1], F32, tag="negm")
                nc.vector.tensor_scalar(
                    negm[:], wvals[:, 0:1], -1.0, None,
                    op0=mybir.AluOpType.mult)
                wexp = sel.tile([128, TOPK], F32, tag="wexp")
                ssum = sel.tile([128, 1], F32, tag="ssum")
                nc.scalar.activation(
                    out=wexp[:], in_=wvals[:, :TOPK],
                    func=mybir.ActivationFunctionType.Exp,
                    bias=negm[:], scale=1.0, accum_out=ssum[:])
                rs = sel.tile([128, 1], F32, tag="rs")
                nc.vector.reciprocal(rs[:], ssum[:])
                wgt = sel.tile([128, TOPK], F32, tag="wgt")
                nc.vector.tensor_scalar(
                    wgt[:], wexp[:], rs[:], None, op0=mybir.AluOpType.mult)

                # ---- gather V rows (one batched indirect DMA), readout ----
                vTg = gat.tile([128, TOPK * 2 * CV], F16, tag="vTg")
                nc.gpsimd.indirect_dma_start(
                    out=vTg[:].rearrange("p (k c) -> p k c", c=2 * CV),
                    out_offset=None, in_=vTb[:],
                    in_offset=bass.IndirectOffsetOnAxis(ap=iku[:], axis=0))
                acc = gat.tile([128, 2 * CV], F32, tag="acc")
                nc.vector.memset(acc[:], 0.0)
                for k in range(TOPK):
                    nc.vector.scalar_tensor_tensor(
                        out=acc[:], in0=vTg[:, k * 2 * CV:(k + 1) * 2 * CV],
                        scalar=wgt[:, k:k + 1], in1=acc[:],
                        op0=mybir.AluOpType.mult, op1=mybir.AluOpType.add)
                nc.sync.dma_start(
                    out=out[t * 128:(t + 1) * 128, :], in_=acc[:])
    nc.finalize()
    _prog_cache["p"] = nc
    return nc


def _host_inputs(qk, mem_k, mem_v1, mem_v2, top_k=TOPK):
    qk = np.asarray(qk, dtype=np.float32)
    mem_k = np.asarray(mem_k, dtype=np.float32)
    mem_v1 = np.asarray(mem_v1, dtype=np.float32)
    mem_v2 = np.asarray(mem_v2, dtype=np.float32)

    q2 = qk.reshape(CK, Q)
    a = np.sum(mem_k[0] * mem_k[0], axis=0, dtype=np.float32)      # [NE]
    na = -0.125 * a
    nh = na.astype(np.float16).astype(np.float32)
    nl = (na - nh).astype(np.float16)
    mkB = np.concatenate(
        [mem_k[0].astype(np.float16), nh.astype(np.float16)[None, :],
         nl[None, :]], axis=0)                                      # [66, NE]
    vTb = np.concatenate(
        [mem_v1[0].T, mem_v2[0].T], axis=1).astype(np.float16)      # [NE, 512]
    mkT32 = np.ascontiguousarray(np.concatenate(
        [mem_k[0].T, na[:, None]], axis=1, dtype=np.float32))      # [NE, 65]
    prow512 = (np.arange(128, dtype=np.float32) * NCH).reshape(128, 1)
    prow192 = (np.arange(128, dtype=np.float32) * NCAND).reshape(128, 1)
    prow24 = (np.arange(128, dtype=np.float32) * 24).reshape(128, 1)
    eps512 = np.broadcast_to(
        np.arange(NCH, dtype=np.float32) * EPS, (128, NCH)).copy()
    eps192 = np.broadcast_to(
        np.arange(NCAND, dtype=np.float32) * EPS, (128, NCAND)).copy()
    eps1536 = np.broadcast_to(
        (np.arange(NSEL * CW, dtype=np.float32) % CW) * EPS,
        (128, NSEL * CW)).copy()

    in_maps = []
    for c in range(NC):
        sl = slice(c * Q_LOC, (c + 1) * Q_LOC)
        qs = 0.25 * q2[:, sl]
        qTb = np.concatenate(
            [qs.astype(np.float16), np.ones((2, Q_LOC), np.float16)],
            axis=0)                                                 # [66, 512]
        qrI = np.ascontiguousarray(np.concatenate(
            [qs.T, np.ones((Q_LOC, 1), np.float32)],
            axis=1, dtype=np.float32))                              # [512, 65]
        in_maps.append({
            "qTb": qTb, "mkB": mkB, "vTb": vTb,
            "mkT32": mkT32, "qrI": qrI,
            "prow512": prow512, "prow192": prow192, "prow24": prow24,
            "eps512": eps512, "eps192": eps192, "eps1536": eps1536,
        })
    return in_maps


def _assemble_output(outs):
    full = np.concatenate(outs, axis=0)
    return np.ascontiguousarray(full.T).reshape(1, 2 * CV, H, W)


def kernel(qk, mem_k, mem_v1, mem_v2, top_k):
    assert int(top_k) == TOPK
    in_maps = _host_inputs(qk, mem_k, mem_v1, mem_v2)
    nc = _build_program()
    res = None
    for attempt in range(3):
        try:
            res = run_bass_kernel_spmd(nc, in_maps, core_ids=list(range(NC)))
            break
        except Exception:
            # transient device-unrecoverable states clear on the next attempt
            if attempt == 2:
                raise
            time.sleep(2.0)
    return _assemble_output([res.results[c]["out"] for c in range(NC)])


# revision 43
# speedup vs baseline: 1.0798x; 1.0798x over previous
import sys, time
sys.path.insert(0, "/opt/trn_rl_repo")
import numpy as np
from concourse import bass, bacc, mybir, tile
from concourse.bass_utils import run_bass_kernel_spmd

# Problem constants (nn_Memory_88656714925588)
B, CK, CV = 1, 64, 256
H, W, T = 64, 64, 8
NE = H * W * T            # 32768 memory elements
Q = H * W * 64 // 64      # 4096 queries
NC = 8                    # cores
Q_LOC = Q // NC           # 512 queries per core (query-sharded)
NQT = Q_LOC // 128        # 4 query tiles per core
TOPK = 20
CW = 64                   # chunk width for the screen
NCH = NE // CW            # 512 chunks per query row
NSEL = 24                 # chunks selected per query (>= 20 guarantees coverage)
NCAND = NSEL * 8          # 192 candidates after per-chunk top-8
NSLICE = 8                # 4096-column slices per tile
SLW = NE // NSLICE        # 4096
F32 = mybir.dt.float32
F16 = mybir.dt.float16
U32 = mybir.dt.uint32
NEG = -1e30
EPS = 2.0 ** -21

_prog_cache = {}


def _build_program():
    if "p" in _prog_cache:
        return _prog_cache["p"]
    nc = bacc.Bacc()
    qTb = nc.dram_tensor("qTb", [CK + 2, Q_LOC], F16, kind="ExternalInput")
    mkB = nc.dram_tensor("mkB", [CK + 2, NE], F16, kind="ExternalInput")
    eps1536 = nc.dram_tensor(
        "eps1536", [128, NSEL * CW], F32, kind="ExternalInput")
    vTb = nc.dram_tensor("vTb", [NE, 2 * CV], F16, kind="ExternalInput")
    mkT32 = nc.dram_tensor("mkT32", [NE, CK + 1], F32, kind="ExternalInput")
    qrI = nc.dram_tensor("qrI", [Q_LOC, CK + 1], F32, kind="ExternalInput")
    prow512 = nc.dram_tensor("prow512", [128, 1], F32, kind="ExternalInput")
    prow192 = nc.dram_tensor("prow192", [128, 1], F32, kind="ExternalInput")
    eps512 = nc.dram_tensor("eps512", [128, NCH], F32, kind="ExternalInput")
    eps192 = nc.dram_tensor("eps192", [128, NCAND], F32, kind="ExternalInput")
    out = nc.dram_tensor("out", [Q_LOC, 2 * CV], F32, kind="ExternalOutput")

    with tile.TileContext(nc) as tc:
        with tc.tile_pool(name="cst", bufs=1) as cst, \
             tc.tile_pool(name="aff", bufs=3) as affp, \
             tc.tile_pool(name="tree", bufs=2) as tre, \
             tc.tile_pool(name="sel", bufs=2) as sel, \
             tc.tile_pool(name="gat", bufs=2) as gat, \
             tc.tile_pool(name="gbig", bufs=1) as gbig, \
             tc.tile_pool(name="psum", bufs=2, space="PSUM") as psum, \
             tc.tile_pool(name="dram", bufs=2, space="DRAM") as dram:

            qt = cst.tile([CK + 2, Q_LOC], F16)
            mkt = cst.tile([CK + 2, NE], F16)
            # small inputs first so the first matmul isn't queued behind
            # the big mk transfers
            nc.sync.dma_start(out=qt[:], in_=qTb[:])
            ep1536 = cst.tile([128, NSEL * CW], F32)
            nc.gpsimd.dma_start(out=ep1536[:], in_=eps1536[:])
            pr512 = cst.tile([128, 1], F32)
            nc.sync.dma_start(out=pr512[:], in_=prow512[:])
            pr192 = cst.tile([128, 1], F32)
            nc.sync.dma_start(out=pr192[:], in_=prow192[:])
            ep512 = cst.tile([128, NCH], F32)
            nc.gpsimd.dma_start(out=ep512[:], in_=eps512[:])
            ep192 = cst.tile([128, NCAND], F32)
            nc.gpsimd.dma_start(out=ep192[:], in_=eps192[:])
            # chunked mk load, split across queues
            for ci in range(NSLICE):
                eng = nc.sync if ci % 2 == 0 else nc.gpsimd
                eng.dma_start(
                    out=mkt[:, ci * SLW:(ci + 1) * SLW],
                    in_=mkB[:, ci * SLW:(ci + 1) * SLW])

            for t in range(NQT):
                qrt = sel.tile([128, CK + 1], F32, tag="qrt")
                nc.sync.dma_start(
                    out=qrt[:], in_=qrI[t * 128:(t + 1) * 128, :])
                affsD = dram.tile([128 * NCH, CW], F16, tag="affsD")
                affsDv = affsD[:].rearrange("(p c) w -> p (c w)", p=128)
                elD = dram.tile([128 * NCAND, 1], F32, tag="elD")
                cmax = tre.tile([128, NCH], F16, tag="cmax")

                for s in range(NSLICE):
                    aff4 = affp.tile([128, SLW], F16, tag="aff4")
                    for h in range(2):
                        ph = psum.tile([128, 2048], F32, tag="ph")
                        for c in range(4):
                            col = s * SLW + h * 2048 + c * 512
                            nc.tensor.matmul(
                                out=ph[:, c * 512:(c + 1) * 512],
                                lhsT=qt[:, t * 128:(t + 1) * 128],
                                rhs=mkt[:, col:col + 512],
                                start=True, stop=True)
                        nc.scalar.activation(
                            out=aff4[:, h * 2048:(h + 1) * 2048], in_=ph[:],
                            func=mybir.ActivationFunctionType.Copy)
                    # stage this slice to DRAM for the per-query rescan gathers
                    eng = nc.gpsimd if s in (2, 5) else nc.sync
                    eng.dma_start(
                        out=affsDv[:, s * SLW:(s + 1) * SLW], in_=aff4[:])
                    # chunk-local pairwise-max tree: 4096 -> 64 chunk maxima
                    a3 = aff4[:].rearrange("p (g w) -> p g w", w=CW)
                    t1 = tre.tile([128, 2048], F16, tag="t1")
                    nc.vector.tensor_tensor(
                        out=t1[:].rearrange("p (g w) -> p g w", w=32),
                        in0=a3[:, :, 0:32], in1=a3[:, :, 32:64],
                        op=mybir.AluOpType.max)
                    t2 = tre.tile([128, 1024], F16, tag="t2")
                    nc.vector.tensor_tensor(
                        out=t2[:].rearrange("p (g w) -> p g w", w=16),
                        in0=t1[:].rearrange("p (g w) -> p g w", w=32)[:, :, 0:16],
                        in1=t1[:].rearrange("p (g w) -> p g w", w=32)[:, :, 16:32],
                        op=mybir.AluOpType.max)
                    t3 = tre.tile([128, 512], F16, tag="t3")
                    nc.vector.tensor_tensor(
                        out=t3[:].rearrange("p (g w) -> p g w", w=8),
                        in0=t2[:].rearrange("p (g w) -> p g w", w=16)[:, :, 0:8],
                        in1=t2[:].rearrange("p (g w) -> p g w", w=16)[:, :, 8:16],
                        op=mybir.AluOpType.max)
                    t4 = tre.tile([128, 256], F16, tag="t4")
                    nc.vector.tensor_tensor(
                        out=t4[:].rearrange("p (g w) -> p g w", w=4),
                        in0=t3[:].rearrange("p (g w) -> p g w", w=8)[:, :, 0:4],
                        in1=t3[:].rearrange("p (g w) -> p g w", w=8)[:, :, 4:8],
                        op=mybir.AluOpType.max)
                    t5 = tre.tile([128, 128], F16, tag="t5")
                    nc.vector.tensor_tensor(
                        out=t5[:].rearrange("p (g w) -> p g w", w=2),
                        in0=t4[:].rearrange("p (g w) -> p g w", w=4)[:, :, 0:2],
                        in1=t4[:].rearrange("p (g w) -> p g w", w=4)[:, :, 2:4],
                        op=mybir.AluOpType.max)
                    nc.vector.tensor_tensor(
                        out=cmax[:, s * 64:(s + 1) * 64],
                        in0=t5[:].rearrange("p (g w) -> p g w", w=2)[:, :, 0],
                        in1=t5[:].rearrange("p (g w) -> p g w", w=2)[:, :, 1],
                        op=mybir.AluOpType.max)

                # ---- select top-NSEL chunks per query (tie-free in f32) ----
                cmaxf = sel.tile([128, NCH], F32, tag="cmaxf")
                nc.vector.tensor_copy(cmaxf[:], cmax[:])
                nc.vector.tensor_tensor(
                    out=cmaxf[:], in0=cmaxf[:], in1=ep512[:],
                    op=mybir.AluOpType.add)
                cidu = sel.tile([128, NSEL], U32, tag="cidu")
                m8 = sel.tile([128, 8], F32, tag="m8")
                for r in range(NSEL // 8):
                    nc.vector.max(out=m8[:], in_=cmaxf[:])
                    nc.vector.max_index(
                        out=cidu[:, r * 8:(r + 1) * 8], in_max=m8[:],
                        in_values=cmaxf[:])
                    if r < NSEL // 8 - 1:
                        nc.vector.match_replace(
                            out=cmaxf[:], in_to_replace=m8[:],
                            in_values=cmaxf[:], imm_value=NEG)
                cidf = sel.tile([128, NSEL], F32, tag="cidf")
                nc.vector.tensor_copy(cidf[:], cidu[:])
                offf = sel.tile([128, NSEL], F32, tag="offf")
                nc.vector.tensor_scalar(
                    offf[:], cidf[:], pr512[:], None, op0=mybir.AluOpType.add)
                offu = sel.tile([128, NSEL], U32, tag="offu")
                nc.vector.tensor_copy(offu[:], offf[:])

                # ---- gather the selected chunks, rescan for top-8 each ----
                g24 = gat.tile([128, NSEL * CW], F16, tag="g24")
                nc.gpsimd.indirect_dma_start(
                    out=g24[:].rearrange("p (k w) -> p k w", w=CW),
                    out_offset=None, in_=affsD[:],
                    in_offset=bass.IndirectOffsetOnAxis(ap=offu[:], axis=0))
                # f32 + positional eps makes the rescan tie-free, so
                # max_index can't alias two tied elements to one position
                g24f = gat.tile([128, NSEL * CW], F32, tag="g24f")
                nc.vector.scalar_tensor_tensor(
                    out=g24f[:], in0=g24[:], scalar=1.0, in1=ep1536[:],
                    op0=mybir.AluOpType.mult, op1=mybir.AluOpType.add)
                cv8 = sel.tile([128, NCAND], F32, tag="cv8")
                pix = sel.tile([128, NCAND], U32, tag="pix")
                for j in range(NSEL):
                    nc.vector.max(
                        out=cv8[:, j * 8:(j + 1) * 8],
                        in_=g24f[:, j * CW:(j + 1) * CW])
                    nc.vector.max_index(
                        out=pix[:, j * 8:(j + 1) * 8],
                        in_max=cv8[:, j * 8:(j + 1) * 8],
                        in_values=g24f[:, j * CW:(j + 1) * CW])
                # decode element index: el = cid*64 + pix
                pixf = sel.tile([128, NCAND], F32, tag="pixf")
                nc.vector.tensor_copy(pixf[:], pix[:])
                elf = sel.tile([128, NCAND], F32, tag="elf")
                nc.vector.scalar_tensor_tensor(
                    out=elf[:].rearrange("p (k r) -> p k r", r=8),
                    in0=cidf[:].rearrange("p (k u) -> p k u", u=1)
                    .broadcast_to([128, NSEL, 8]),
                    scalar=float(CW),
                    in1=pixf[:].rearrange("p (k r) -> p k r", r=8),
                    op0=mybir.AluOpType.mult, op1=mybir.AluOpType.add)
                nc.sync.dma_start(
                    out=elD[:].rearrange("(p u) one -> p (u one)", p=128),
                    in_=elf[:])

                # ---- merge: exact top-20 of the 192 candidates ----
                cvf = sel.tile([128, NCAND], F32, tag="cvf")
                nc.vector.tensor_tensor(
                    out=cvf[:], in0=cv8[:], in1=ep192[:],
                    op=mybir.AluOpType.add)
                gvals = sel.tile([128, 24], F32, tag="gvals")
                gpos = sel.tile([128, 24], U32, tag="gpos")
                for r in range(3):
                    g8 = gvals[:, r * 8:(r + 1) * 8]
                    nc.vector.max(out=g8, in_=cvf[:])
                    nc.vector.max_index(
                        out=gpos[:, r * 8:(r + 1) * 8], in_max=g8,
                        in_values=cvf[:])
                    if r < 2:
                        nc.vector.match_replace(
                            out=cvf[:], in_to_replace=g8, in_values=cvf[:],
                            imm_value=NEG)
                # ---- exact fp32 rescore of the 24 candidates ----
                gposf = sel.tile([128, 24], F32, tag="gposf")
                nc.vector.tensor_copy(gposf[:], gpos[:])
                off2 = sel.tile([128, 24], F32, tag="off2")
                nc.vector.tensor_scalar(
                    off2[:], gposf[:], pr192[:], None, op0=mybir.AluOpType.add)
                offu2 = sel.tile([128, 24], U32, tag="offu2")
                nc.vector.tensor_copy(offu2[:], off2[:])
                el24 = sel.tile([128, 24], F32, tag="el24")
                nc.gpsimd.indirect_dma_start(
                    out=el24[:], out_offset=None, in_=elD[:],
                    in_offset=bass.IndirectOffsetOnAxis(ap=offu2[:], axis=0))
                el24u = sel.tile([128, 24], U32, tag="el24u")
                nc.vector.tensor_copy(el24u[:], el24[:])
                gmk = gbig.tile([128, 24 * (CK + 1)], F32, tag="gmk")
                nc.gpsimd.indirect_dma_start(
                    out=gmk[:].rearrange("p (k c) -> p k c", c=CK + 1),
                    out_offset=None, in_=mkT32[:],
                    in_offset=bass.IndirectOffsetOnAxis(ap=el24u[:], axis=0))
                # V rows for all 24 candidates, issued early so the gather
                # overlaps the rescore; non-top-20 slots get zero weight
                vTg = gbig.tile([128, 24 * 2 * CV], F16, tag="vTg")
                nc.gpsimd.indirect_dma_start(
                    out=vTg[:].rearrange("p (k c) -> p k c", c=2 * CV),
                    out_offset=None, in_=vTb[:],
                    in_offset=bass.IndirectOffsetOnAxis(ap=el24u[:], axis=0))
                nc.vector.tensor_tensor(
                    out=gmk[:].rearrange("p (k c) -> p k c", c=CK + 1),
                    in0=gmk[:].rearrange("p (k c) -> p k c", c=CK + 1),
                    in1=qrt[:].rearrange("p (u c) -> p u c", u=1)
                    .broadcast_to([128, 24, CK + 1]),
                    op=mybir.AluOpType.mult)
                av24 = sel.tile([128, 24], F32, tag="av24")
                nc.vector.tensor_reduce(
                    out=av24[:],
                    in_=gmk[:].rearrange("p (k c) -> p k c", c=CK + 1),
                    axis=mybir.AxisListType.X, op=mybir.AluOpType.add)
                nc.vector.tensor_tensor(
                    out=av24[:], in0=av24[:], in1=ep192[:, :24],
                    op=mybir.AluOpType.add)
                # ranked values (top-20 threshold), no positions needed
                av24c = sel.tile([128, 24], F32, tag="av24c")
                nc.vector.tensor_copy(av24c[:], av24[:])
                wvals = sel.tile([128, 24], F32, tag="wvals")
                for r in range(3):
                    w8 = wvals[:, r * 8:(r + 1) * 8]
                    nc.vector.max(out=w8, in_=av24c[:])
                    if r < 2:
                        nc.vector.match_replace(
                            out=av24c[:], in_to_replace=w8, in_values=av24c[:],
                            imm_value=NEG)

                # ---- masked softmax over all 24 slots (ranks >= 20 -> 0) ---
                mask = sel.tile([128, 24], F32, tag="mask")
                nc.vector.tensor_scalar(
                    mask[:], av24[:], wvals[:, 19:20], None,
                    op0=mybir.AluOpType.is_ge)
                negm40 = sel.tile([128, 1], F32, tag="negm40")
                nc.vector.tensor_scalar(
                    negm40[:], wvals[:, 0:1], -1.0, -40.0,
                    op0=mybir.AluOpType.mult, op1=mybir.AluOpType.add)
                # masked slots sit 40 below the kept ones -> exp ~ 0
                dms = sel.tile([128, 24], F32, tag="dms")
                nc.vector.scalar_tensor_tensor(
                    out=dms[:], in0=mask[:], scalar=40.0, in1=av24[:],
                    op0=mybir.AluOpType.mult, op1=mybir.AluOpType.add)
                wexp = sel.tile([128, 24], F32, tag="wexp")
                ssum = sel.tile([128, 1], F32, tag="ssum")
                nc.scalar.activation(
                    out=wexp[:], in_=dms[:],
                    func=mybir.ActivationFunctionType.Exp,
                    bias=negm40[:], scale=1.0, accum_out=ssum[:])
                rs = sel.tile([128, 1], F32, tag="rs")
                nc.vector.reciprocal(rs[:], ssum[:])
                wgt = sel.tile([128, 24], F32, tag="wgt")
                nc.vector.tensor_scalar(
                    wgt[:], wexp[:], rs[:], None, op0=mybir.AluOpType.mult)

                # ---- weighted readout over the 24 gathered V rows ----
                acc = gat.tile([128, 2 * CV], F32, tag="acc")
                nc.vector.memset(acc[:], 0.0)
                for k in range(24):
                    nc.vector.scalar_tensor_tensor(
                        out=acc[:], in0=vTg[:, k * 2 * CV:(k + 1) * 2 * CV],
                        scalar=wgt[:, k:k + 1], in1=acc[:],
                        op0=mybir.AluOpType.mult, op1=mybir.AluOpType.add)
                nc.sync.dma_start(
                    out=out[t * 128:(t + 1) * 128, :], in_=acc[:])
    nc.finalize()
    _prog_cache["p"] = nc
    return nc


def _host_inputs(qk, mem_k, mem_v1, mem_v2, top_k=TOPK):
    qk = np.asarray(qk, dtype=np.float32)
    mem_k = np.asarray(mem_k, dtype=np.float32)
    mem_v1 = np.asarray(mem_v1, dtype=np.float32)
    mem_v2 = np.asarray(mem_v2, dtype=np.float32)

    q2 = qk.reshape(CK, Q)
    a = np.sum(mem_k[0] * mem_k[0], axis=0, dtype=np.float32)      # [NE]
    na = -0.125 * a
    nh = na.astype(np.float16).astype(np.float32)
    nl = (na - nh).astype(np.float16)
    mkB = np.concatenate(
        [mem_k[0].astype(np.float16), nh.astype(np.float16)[None, :],
         nl[None, :]], axis=0)                                      # [66, NE]
    vTb = np.concatenate(
        [mem_v1[0].T, mem_v2[0].T], axis=1).astype(np.float16)      # [NE, 512]
    mkT32 = np.ascontiguousarray(np.concatenate(
        [mem_k[0].T, na[:, None]], axis=1, dtype=np.float32))      # [NE, 65]
    prow512 = (np.arange(128, dtype=np.float32) * NCH).reshape(128, 1)
    prow192 = (np.arange(128, dtype=np.float32) * NCAND).reshape(128, 1)
    eps512 = np.broadcast_to(
        np.arange(NCH, dtype=np.float32) * EPS, (128, NCH)).copy()
    eps192 = np.broadcast_to(
        np.arange(NCAND, dtype=np.float32) * EPS, (128, NCAND)).copy()
    eps1536 = np.broadcast_to(
        (np.arange(NSEL * CW, dtype=np.float32) % CW) * EPS,
        (128, NSEL * CW)).copy()

    in_maps = []
    for c in range(NC):
        sl = slice(c * Q_LOC, (c + 1) * Q_LOC)
        qs = 0.25 * q2[:, sl]
        qTb = np.concatenate(
            [qs.astype(np.float16), np.ones((2, Q_LOC), np.float16)],
            axis=0)                                                 # [66, 512]
        qrI = np.ascontiguousarray(np.concatenate(
            [qs.T, np.ones((Q_LOC, 1), np.float32)],
            axis=1, dtype=np.float32))                              # [512, 65]
        in_maps.append({
            "qTb": qTb, "mkB": mkB, "vTb": vTb,
            "mkT32": mkT32, "qrI": qrI,
            "prow512": prow512, "prow192": prow192,
            "eps512": eps512, "eps192": eps192, "eps1536": eps1536,
        })
    return in_maps


def _assemble_output(outs):
    full = np.concatenate(outs, axis=0)
    return np.ascontiguousarray(full.T).reshape(1, 2 * CV, H, W)


def kernel(qk, mem_k, mem_v1, mem_v2, top_k):
    assert int(top_k) == TOPK
    in_maps = _host_inputs(qk, mem_k, mem_v1, mem_v2)
    nc = _build_program()
    res = None
    for attempt in range(3):
        try:
            res = run_bass_kernel_spmd(nc, in_maps, core_ids=list(range(NC)))
            break
        except Exception:
            # transient device-unrecoverable states clear on the next attempt
            if attempt == 2:
                raise
            time.sleep(2.0)
    return _assemble_output([res.results[c]["out"] for c in range(NC)])


# revision 46
# speedup vs baseline: 1.0842x; 1.0041x over previous
import sys, time
sys.path.insert(0, "/opt/trn_rl_repo")
import numpy as np
from concourse import bass, bacc, mybir, tile
from concourse.bass_utils import run_bass_kernel_spmd

# Problem constants (nn_Memory_88656714925588)
B, CK, CV = 1, 64, 256
H, W, T = 64, 64, 8
NE = H * W * T            # 32768 memory elements
Q = H * W * 64 // 64      # 4096 queries
NC = 8                    # cores
Q_LOC = Q // NC           # 512 queries per core (query-sharded)
NQT = Q_LOC // 128        # 4 query tiles per core
TOPK = 20
CW = 64                   # chunk width for the screen
NCH = NE // CW            # 512 chunks per query row
NSEL = 24                 # chunks selected per query (>= 20 guarantees coverage)
NCAND = NSEL * 8          # 192 candidates after per-chunk top-8
NSLICE = 8                # 4096-column slices per tile
SLW = NE // NSLICE        # 4096
F32 = mybir.dt.float32
F16 = mybir.dt.float16
U32 = mybir.dt.uint32
NEG = -1e30
EPS = 2.0 ** -21

_prog_cache = {}


def _build_program():
    if "p" in _prog_cache:
        return _prog_cache["p"]
    nc = bacc.Bacc()
    qTb = nc.dram_tensor("qTb", [CK + 2, Q_LOC], F16, kind="ExternalInput")
    mkB = nc.dram_tensor("mkB", [CK + 2, NE], F16, kind="ExternalInput")
    eps1536 = nc.dram_tensor(
        "eps1536", [128, NSEL * CW], F32, kind="ExternalInput")
    vTb = nc.dram_tensor("vTb", [NE, 2 * CV], F16, kind="ExternalInput")
    mkT32 = nc.dram_tensor("mkT32", [NE, CK + 1], F32, kind="ExternalInput")
    qrI = nc.dram_tensor("qrI", [Q_LOC, CK + 1], F32, kind="ExternalInput")
    prow512 = nc.dram_tensor("prow512", [128, 1], F32, kind="ExternalInput")
    prow192 = nc.dram_tensor("prow192", [128, 1], F32, kind="ExternalInput")
    eps512 = nc.dram_tensor("eps512", [128, NCH], F32, kind="ExternalInput")
    eps192 = nc.dram_tensor("eps192", [128, NCAND], F32, kind="ExternalInput")
    out = nc.dram_tensor("out", [Q_LOC, 2 * CV], F32, kind="ExternalOutput")

    with tile.TileContext(nc) as tc:
        with tc.tile_pool(name="cst", bufs=1) as cst, \
             tc.tile_pool(name="aff", bufs=3) as affp, \
             tc.tile_pool(name="tree", bufs=2) as tre, \
             tc.tile_pool(name="sel", bufs=2) as sel, \
             tc.tile_pool(name="gat", bufs=2) as gat, \
             tc.tile_pool(name="gbig", bufs=1) as gbig, \
             tc.tile_pool(name="psum", bufs=2, space="PSUM") as psum, \
             tc.tile_pool(name="dram", bufs=2, space="DRAM") as dram:

            qt = cst.tile([CK + 2, Q_LOC], F16)
            mkt = cst.tile([CK + 2, NE], F16)
            # small inputs first so the first matmul isn't queued behind
            # the big mk transfers
            nc.sync.dma_start(out=qt[:], in_=qTb[:])
            ep1536 = cst.tile([128, NSEL * CW], F32)
            nc.gpsimd.dma_start(out=ep1536[:], in_=eps1536[:])
            pr512 = cst.tile([128, 1], F32)
            nc.sync.dma_start(out=pr512[:], in_=prow512[:])
            pr192 = cst.tile([128, 1], F32)
            nc.sync.dma_start(out=pr192[:], in_=prow192[:])
            ep512 = cst.tile([128, NCH], F32)
            nc.gpsimd.dma_start(out=ep512[:], in_=eps512[:])
            ep192 = cst.tile([128, NCAND], F32)
            nc.gpsimd.dma_start(out=ep192[:], in_=eps192[:])
            # chunked mk load, split across queues
            for ci in range(NSLICE):
                eng = nc.sync if ci % 2 == 0 else nc.gpsimd
                eng.dma_start(
                    out=mkt[:, ci * SLW:(ci + 1) * SLW],
                    in_=mkB[:, ci * SLW:(ci + 1) * SLW])

            for t in range(NQT):
                qrt = sel.tile([128, CK + 1], F32, tag="qrt")
                nc.sync.dma_start(
                    out=qrt[:], in_=qrI[t * 128:(t + 1) * 128, :])
                affsD = dram.tile([128 * NCH, CW], F16, tag="affsD")
                affsDv = affsD[:].rearrange("(p c) w -> p (c w)", p=128)
                elD = dram.tile([128 * NCAND, 1], F32, tag="elD")
                cmax = tre.tile([128, NCH], F16, tag="cmax")

                for s in range(NSLICE):
                    aff4 = affp.tile([128, SLW], F16, tag="aff4")
                    for h in range(2):
                        ph = psum.tile([128, 2048], F32, tag="ph")
                        for c in range(4):
                            col = s * SLW + h * 2048 + c * 512
                            nc.tensor.matmul(
                                out=ph[:, c * 512:(c + 1) * 512],
                                lhsT=qt[:, t * 128:(t + 1) * 128],
                                rhs=mkt[:, col:col + 512],
                                start=True, stop=True)
                        nc.scalar.activation(
                            out=aff4[:, h * 2048:(h + 1) * 2048], in_=ph[:],
                            func=mybir.ActivationFunctionType.Copy)
                    # stage this slice to DRAM for the per-query rescan gathers
                    eng = nc.gpsimd if s in (2, 5) else nc.sync
                    eng.dma_start(
                        out=affsDv[:, s * SLW:(s + 1) * SLW], in_=aff4[:])
                    # chunk-local pairwise-max tree: 4096 -> 64 chunk maxima
                    a3 = aff4[:].rearrange("p (g w) -> p g w", w=CW)
                    t1 = tre.tile([128, 2048], F16, tag="t1")
                    nc.vector.tensor_tensor(
                        out=t1[:].rearrange("p (g w) -> p g w", w=32),
                        in0=a3[:, :, 0:32], in1=a3[:, :, 32:64],
                        op=mybir.AluOpType.max)
                    t2 = tre.tile([128, 1024], F16, tag="t2")
                    nc.vector.tensor_tensor(
                        out=t2[:].rearrange("p (g w) -> p g w", w=16),
                        in0=t1[:].rearrange("p (g w) -> p g w", w=32)[:, :, 0:16],
                        in1=t1[:].rearrange("p (g w) -> p g w", w=32)[:, :, 16:32],
                        op=mybir.AluOpType.max)
                    t3 = tre.tile([128, 512], F16, tag="t3")
                    nc.vector.tensor_tensor(
                        out=t3[:].rearrange("p (g w) -> p g w", w=8),
                        in0=t2[:].rearrange("p (g w) -> p g w", w=16)[:, :, 0:8],
                        in1=t2[:].rearrange("p (g w) -> p g w", w=16)[:, :, 8:16],
                        op=mybir.AluOpType.max)
                    t4 = tre.tile([128, 256], F16, tag="t4")
                    nc.vector.tensor_tensor(
                        out=t4[:].rearrange("p (g w) -> p g w", w=4),
                        in0=t3[:].rearrange("p (g w) -> p g w", w=8)[:, :, 0:4],
                        in1=t3[:].rearrange("p (g w) -> p g w", w=8)[:, :, 4:8],
                        op=mybir.AluOpType.max)
                    t5 = tre.tile([128, 128], F16, tag="t5")
                    nc.vector.tensor_tensor(
                        out=t5[:].rearrange("p (g w) -> p g w", w=2),
                        in0=t4[:].rearrange("p (g w) -> p g w", w=4)[:, :, 0:2],
                        in1=t4[:].rearrange("p (g w) -> p g w", w=4)[:, :, 2:4],
                        op=mybir.AluOpType.max)
                    nc.vector.tensor_tensor(
                        out=cmax[:, s * 64:(s + 1) * 64],
                        in0=t5[:].rearrange("p (g w) -> p g w", w=2)[:, :, 0],
                        in1=t5[:].rearrange("p (g w) -> p g w", w=2)[:, :, 1],
                        op=mybir.AluOpType.max)

                # ---- select top-NSEL chunks per query (tie-free in f32) ----
                cmaxf = sel.tile([128, NCH], F32, tag="cmaxf")
                nc.vector.scalar_tensor_tensor(
                    out=cmaxf[:], in0=cmax[:], scalar=1.0, in1=ep512[:],
                    op0=mybir.AluOpType.mult, op1=mybir.AluOpType.add)
                cidu = sel.tile([128, NSEL], U32, tag="cidu")
                m8 = sel.tile([128, 8], F32, tag="m8")
                for r in range(NSEL // 8):
                    nc.vector.max(out=m8[:], in_=cmaxf[:])
                    nc.vector.max_index(
                        out=cidu[:, r * 8:(r + 1) * 8], in_max=m8[:],
                        in_values=cmaxf[:])
                    if r < NSEL // 8 - 1:
                        nc.vector.match_replace(
                            out=cmaxf[:], in_to_replace=m8[:],
                            in_values=cmaxf[:], imm_value=NEG)
                cidf = sel.tile([128, NSEL], F32, tag="cidf")
                nc.vector.tensor_copy(cidf[:], cidu[:])
                offf = sel.tile([128, NSEL], F32, tag="offf")
                nc.vector.tensor_scalar(
                    offf[:], cidf[:], pr512[:], None, op0=mybir.AluOpType.add)
                offu = sel.tile([128, NSEL], U32, tag="offu")
                nc.vector.tensor_copy(offu[:], offf[:])

                # ---- gather the selected chunks, rescan for top-8 each ----
                g24 = gat.tile([128, NSEL * CW], F16, tag="g24")
                nc.gpsimd.indirect_dma_start(
                    out=g24[:].rearrange("p (k w) -> p k w", w=CW),
                    out_offset=None, in_=affsD[:],
                    in_offset=bass.IndirectOffsetOnAxis(ap=offu[:], axis=0))
                # f32 + positional eps makes the rescan tie-free, so
                # max_index can't alias two tied elements to one position
                g24f = gat.tile([128, NSEL * CW], F32, tag="g24f")
                nc.vector.scalar_tensor_tensor(
                    out=g24f[:], in0=g24[:], scalar=1.0, in1=ep1536[:],
                    op0=mybir.AluOpType.mult, op1=mybir.AluOpType.add)
                cv8 = sel.tile([128, NCAND], F32, tag="cv8")
                pix = sel.tile([128, NCAND], U32, tag="pix")
                for j in range(NSEL):
                    nc.vector.max(
                        out=cv8[:, j * 8:(j + 1) * 8],
                        in_=g24f[:, j * CW:(j + 1) * CW])
                    nc.vector.max_index(
                        out=pix[:, j * 8:(j + 1) * 8],
                        in_max=cv8[:, j * 8:(j + 1) * 8],
                        in_values=g24f[:, j * CW:(j + 1) * CW])
                # decode element index: el = cid*64 + pix
                pixf = sel.tile([128, NCAND], F32, tag="pixf")
                nc.vector.tensor_copy(pixf[:], pix[:])
                elf = sel.tile([128, NCAND], F32, tag="elf")
                nc.vector.scalar_tensor_tensor(
                    out=elf[:].rearrange("p (k r) -> p k r", r=8),
                    in0=cidf[:].rearrange("p (k u) -> p k u", u=1)
                    .broadcast_to([128, NSEL, 8]),
                    scalar=float(CW),
                    in1=pixf[:].rearrange("p (k r) -> p k r", r=8),
                    op0=mybir.AluOpType.mult, op1=mybir.AluOpType.add)
                nc.sync.dma_start(
                    out=elD[:].rearrange("(p u) one -> p (u one)", p=128),
                    in_=elf[:])

                # ---- merge: exact top-20 of the 192 candidates ----
                cvf = sel.tile([128, NCAND], F32, tag="cvf")
                nc.vector.tensor_tensor(
                    out=cvf[:], in0=cv8[:], in1=ep192[:],
                    op=mybir.AluOpType.add)
                gvals = sel.tile([128, 24], F32, tag="gvals")
                gpos = sel.tile([128, 24], U32, tag="gpos")
                for r in range(3):
                    g8 = gvals[:, r * 8:(r + 1) * 8]
                    nc.vector.max(out=g8, in_=cvf[:])
                    nc.vector.max_index(
                        out=gpos[:, r * 8:(r + 1) * 8], in_max=g8,
                        in_values=cvf[:])
                    if r < 2:
                        nc.vector.match_replace(
                            out=cvf[:], in_to_replace=g8, in_values=cvf[:],
                            imm_value=NEG)
                # ---- exact fp32 rescore of the 24 candidates ----
                gposf = sel.tile([128, 24], F32, tag="gposf")
                nc.vector.tensor_copy(gposf[:], gpos[:])
                off2 = sel.tile([128, 24], F32, tag="off2")
                nc.vector.tensor_scalar(
                    off2[:], gposf[:], pr192[:], None, op0=mybir.AluOpType.add)
                offu2 = sel.tile([128, 24], U32, tag="offu2")
                nc.vector.tensor_copy(offu2[:], off2[:])
                el24 = sel.tile([128, 24], F32, tag="el24")
                nc.gpsimd.indirect_dma_start(
                    out=el24[:], out_offset=None, in_=elD[:],
                    in_offset=bass.IndirectOffsetOnAxis(ap=offu2[:], axis=0))
                el24u = sel.tile([128, 24], U32, tag="el24u")
                nc.vector.tensor_copy(el24u[:], el24[:])
                gmk = gbig.tile([128, 24 * (CK + 1)], F32, tag="gmk")
                nc.gpsimd.indirect_dma_start(
                    out=gmk[:].rearrange("p (k c) -> p k c", c=CK + 1),
                    out_offset=None, in_=mkT32[:],
                    in_offset=bass.IndirectOffsetOnAxis(ap=el24u[:], axis=0))
                # V rows for all 24 candidates, issued early so the gather
                # overlaps the rescore; non-top-20 slots get zero weight
                vTg = gbig.tile([128, 24 * 2 * CV], F16, tag="vTg")
                nc.gpsimd.indirect_dma_start(
                    out=vTg[:].rearrange("p (k c) -> p k c", c=2 * CV),
                    out_offset=None, in_=vTb[:],
                    in_offset=bass.IndirectOffsetOnAxis(ap=el24u[:], axis=0))
                nc.vector.tensor_tensor(
                    out=gmk[:].rearrange("p (k c) -> p k c", c=CK + 1),
                    in0=gmk[:].rearrange("p (k c) -> p k c", c=CK + 1),
                    in1=qrt[:].rearrange("p (u c) -> p u c", u=1)
                    .broadcast_to([128, 24, CK + 1]),
                    op=mybir.AluOpType.mult)
                av24 = sel.tile([128, 24], F32, tag="av24")
                nc.vector.tensor_reduce(
                    out=av24[:],
                    in_=gmk[:].rearrange("p (k c) -> p k c", c=CK + 1),
                    axis=mybir.AxisListType.X, op=mybir.AluOpType.add)
                nc.vector.tensor_tensor(
                    out=av24[:], in0=av24[:], in1=ep192[:, :24],
                    op=mybir.AluOpType.add)
                # ranked values (top-20 threshold), no positions needed
                av24c = sel.tile([128, 24], F32, tag="av24c")
                nc.vector.tensor_copy(av24c[:], av24[:])
                wvals = sel.tile([128, 24], F32, tag="wvals")
                for r in range(3):
                    w8 = wvals[:, r * 8:(r + 1) * 8]
                    nc.vector.max(out=w8, in_=av24c[:])
                    if r < 2:
                        nc.vector.match_replace(
                            out=av24c[:], in_to_replace=w8, in_values=av24c[:],
                            imm_value=NEG)

                # ---- masked softmax over all 24 slots (ranks >= 20 -> 0) ---
                mask = sel.tile([128, 24], F32, tag="mask")
                nc.vector.tensor_scalar(
                    mask[:], av24[:], wvals[:, 19:20], None,
                    op0=mybir.AluOpType.is_ge)
                negm40 = sel.tile([128, 1], F32, tag="negm40")
                nc.vector.tensor_scalar(
                    negm40[:], wvals[:, 0:1], -1.0, -40.0,
                    op0=mybir.AluOpType.mult, op1=mybir.AluOpType.add)
                # masked slots sit 40 below the kept ones -> exp ~ 0
                dms = sel.tile([128, 24], F32, tag="dms")
                nc.vector.scalar_tensor_tensor(
                    out=dms[:], in0=mask[:], scalar=40.0, in1=av24[:],
                    op0=mybir.AluOpType.mult, op1=mybir.AluOpType.add)
                wexp = sel.tile([128, 24], F32, tag="wexp")
                ssum = sel.tile([128, 1], F32, tag="ssum")
                nc.scalar.activation(
                    out=wexp[:], in_=dms[:],
                    func=mybir.ActivationFunctionType.Exp,
                    bias=negm40[:], scale=1.0, accum_out=ssum[:])
                rs = sel.tile([128, 1], F32, tag="rs")
                nc.vector.reciprocal(rs[:], ssum[:])
                wgt = sel.tile([128, 24], F32, tag="wgt")
                nc.vector.tensor_scalar(
                    wgt[:], wexp[:], rs[:], None, op0=mybir.AluOpType.mult)

                # ---- weighted readout over the 24 gathered V rows ----
                acc = gat.tile([128, 2 * CV], F32, tag="acc")
                nc.vector.memset(acc[:], 0.0)
                for k in range(24):
                    nc.vector.scalar_tensor_tensor(
                        out=acc[:], in0=vTg[:, k * 2 * CV:(k + 1) * 2 * CV],
                        scalar=wgt[:, k:k + 1], in1=acc[:],
                        op0=mybir.AluOpType.mult, op1=mybir.AluOpType.add)
                nc.sync.dma_start(
                    out=out[t * 128:(t + 1) * 128, :], in_=acc[:])
    nc.finalize()
    _prog_cache["p"] = nc
    return nc


def _host_inputs(qk, mem_k, mem_v1, mem_v2, top_k=TOPK):
    qk = np.asarray(qk, dtype=np.float32)
    mem_k = np.asarray(mem_k, dtype=np.float32)
    mem_v1 = np.asarray(mem_v1, dtype=np.float32)
    mem_v2 = np.asarray(mem_v2, dtype=np.float32)

    q2 = qk.reshape(CK, Q)
    a = np.sum(mem_k[0] * mem_k[0], axis=0, dtype=np.float32)      # [NE]
    na = -0.125 * a
    nh = na.astype(np.float16).astype(np.float32)
    nl = (na - nh).astype(np.float16)
    mkB = np.concatenate(
        [mem_k[0].astype(np.float16), nh.astype(np.float16)[None, :],
         nl[None, :]], axis=0)                                      # [66, NE]
    vTb = np.concatenate(
        [mem_v1[0].T, mem_v2[0].T], axis=1).astype(np.float16)      # [NE, 512]
    mkT32 = np.ascontiguousarray(np.concatenate(
        [mem_k[0].T, na[:, None]], axis=1, dtype=np.float32))      # [NE, 65]
    prow512 = (np.arange(128, dtype=np.float32) * NCH).reshape(128, 1)
    prow192 = (np.arange(128, dtype=np.float32) * NCAND).reshape(128, 1)
    eps512 = np.broadcast_to(
        np.arange(NCH, dtype=np.float32) * EPS, (128, NCH)).copy()
    eps192 = np.broadcast_to(
        np.arange(NCAND, dtype=np.float32) * EPS, (128, NCAND)).copy()
    eps1536 = np.broadcast_to(
        (np.arange(NSEL * CW, dtype=np.float32) % CW) * EPS,
        (128, NSEL * CW)).copy()

    in_maps = []
    for c in range(NC):
        sl = slice(c * Q_LOC, (c + 1) * Q_LOC)
        qs = 0.25 * q2[:, sl]
        qTb = np.concatenate(
            [qs.astype(np.float16), np.ones((2, Q_LOC), np.float16)],
            axis=0)                                                 # [66, 512]
        qrI = np.ascontiguousarray(np.concatenate(
            [qs.T, np.ones((Q_LOC, 1), np.float32)],
            axis=1, dtype=np.float32))                              # [512, 65]
        in_maps.append({
            "qTb": qTb, "mkB": mkB, "vTb": vTb,
            "mkT32": mkT32, "qrI": qrI,
            "prow512": prow512, "prow192": prow192,
            "eps512": eps512, "eps192": eps192, "eps1536": eps1536,
        })
    return in_maps


def _assemble_output(outs):
    full = np.concatenate(outs, axis=0)
    return np.ascontiguousarray(full.T).reshape(1, 2 * CV, H, W)


def kernel(qk, mem_k, mem_v1, mem_v2, top_k):
    assert int(top_k) == TOPK
    in_maps = _host_inputs(qk, mem_k, mem_v1, mem_v2)
    nc = _build_program()
    res = None
    for attempt in range(3):
        try:
            res = run_bass_kernel_spmd(nc, in_maps, core_ids=list(range(NC)))
            break
        except Exception:
            # transient device-unrecoverable states clear on the next attempt
            if attempt == 2:
                raise
            time.sleep(2.0)
    return _assemble_output([res.results[c]["out"] for c in range(NC)])


# revision 48
# speedup vs baseline: 1.1088x; 1.0227x over previous
import sys, time
sys.path.insert(0, "/opt/trn_rl_repo")
import numpy as np
from concourse import bass, bacc, mybir, tile
from concourse.bass_utils import run_bass_kernel_spmd

# Problem constants (nn_Memory_88656714925588)
B, CK, CV = 1, 64, 256
H, W, T = 64, 64, 8
NE = H * W * T            # 32768 memory elements
Q = H * W * 64 // 64      # 4096 queries
NC = 8                    # cores
Q_LOC = Q // NC           # 512 queries per core (query-sharded)
NQT = Q_LOC // 128        # 4 query tiles per core
TOPK = 20
CW = 64                   # chunk width for the screen
NCH = NE // CW            # 512 chunks per query row
NSEL = 24                 # chunks selected per query (>= 20 guarantees coverage)
NCAND = NSEL * 8          # 192 candidates after per-chunk top-8
NSLICE = 8                # 4096-column slices per tile
SLW = NE // NSLICE        # 4096
F32 = mybir.dt.float32
F16 = mybir.dt.float16
U32 = mybir.dt.uint32
NEG = -1e30
EPS = 2.0 ** -21

_prog_cache = {}


def _build_program():
    if "p" in _prog_cache:
        return _prog_cache["p"]
    nc = bacc.Bacc()
    qTb = nc.dram_tensor("qTb", [CK + 2, Q_LOC], F16, kind="ExternalInput")
    mkB = nc.dram_tensor("mkB", [CK + 2, NE], F16, kind="ExternalInput")
    eps1536 = nc.dram_tensor(
        "eps1536", [128, NSEL * CW], F32, kind="ExternalInput")
    vTb = nc.dram_tensor("vTb", [NE, 2 * CV], F16, kind="ExternalInput")
    mkT32 = nc.dram_tensor("mkT32", [NE, CK + 1], F32, kind="ExternalInput")
    qrI = nc.dram_tensor("qrI", [Q_LOC, CK + 1], F32, kind="ExternalInput")
    prow512 = nc.dram_tensor("prow512", [128, 1], F32, kind="ExternalInput")
    prow192 = nc.dram_tensor("prow192", [128, 1], F32, kind="ExternalInput")
    eps512 = nc.dram_tensor("eps512", [128, NCH], F32, kind="ExternalInput")
    eps192 = nc.dram_tensor("eps192", [128, NCAND], F32, kind="ExternalInput")
    out = nc.dram_tensor("out", [Q_LOC, 2 * CV], F32, kind="ExternalOutput")

    with tile.TileContext(nc) as tc:
        with tc.tile_pool(name="cst", bufs=1) as cst, \
             tc.tile_pool(name="aff", bufs=3) as affp, \
             tc.tile_pool(name="tree", bufs=2) as tre, \
             tc.tile_pool(name="sel", bufs=2) as sel, \
             tc.tile_pool(name="gat", bufs=2) as gat, \
             tc.tile_pool(name="gbig", bufs=1) as gbig, \
             tc.tile_pool(name="psum", bufs=2, space="PSUM") as psum, \
             tc.tile_pool(name="dram", bufs=2, space="DRAM") as dram:

            qt = cst.tile([CK + 2, Q_LOC], F16)
            mkt = cst.tile([CK + 2, NE], F16)
            # small inputs first so the first matmul isn't queued behind
            # the big mk transfers
            nc.sync.dma_start(out=qt[:], in_=qTb[:])
            ep1536 = cst.tile([128, NSEL * CW], F32)
            nc.gpsimd.dma_start(out=ep1536[:], in_=eps1536[:])
            pr512 = cst.tile([128, 1], F32)
            nc.sync.dma_start(out=pr512[:], in_=prow512[:])
            pr192 = cst.tile([128, 1], F32)
            nc.sync.dma_start(out=pr192[:], in_=prow192[:])
            ep512 = cst.tile([128, NCH], F32)
            nc.gpsimd.dma_start(out=ep512[:], in_=eps512[:])
            ep192 = cst.tile([128, NCAND], F32)
            nc.gpsimd.dma_start(out=ep192[:], in_=eps192[:])
            # chunked mk load, split across queues; tiny first chunk so the
            # first matmul can start almost immediately
            bounds = [0, 512, 1024, 2048, 4096] + \
                [SLW * i for i in range(2, NSLICE + 1)]
            for ci in range(len(bounds) - 1):
                eng = nc.sync if ci % 2 == 0 else nc.gpsimd
                eng.dma_start(
                    out=mkt[:, bounds[ci]:bounds[ci + 1]],
                    in_=mkB[:, bounds[ci]:bounds[ci + 1]])

            for t in range(NQT):
                qrt = sel.tile([128, CK + 1], F32, tag="qrt")
                nc.sync.dma_start(
                    out=qrt[:], in_=qrI[t * 128:(t + 1) * 128, :])
                affsD = dram.tile([128 * NCH, CW], F16, tag="affsD")
                affsDv = affsD[:].rearrange("(p c) w -> p (c w)", p=128)
                elD = dram.tile([128 * NCAND, 1], F32, tag="elD")
                cmax = tre.tile([128, NCH], F16, tag="cmax")

                for s in range(NSLICE):
                    aff4 = affp.tile([128, SLW], F16, tag="aff4")
                    for h in range(2):
                        ph = psum.tile([128, 2048], F32, tag="ph")
                        for c in range(4):
                            col = s * SLW + h * 2048 + c * 512
                            nc.tensor.matmul(
                                out=ph[:, c * 512:(c + 1) * 512],
                                lhsT=qt[:, t * 128:(t + 1) * 128],
                                rhs=mkt[:, col:col + 512],
                                start=True, stop=True)
                        nc.scalar.activation(
                            out=aff4[:, h * 2048:(h + 1) * 2048], in_=ph[:],
                            func=mybir.ActivationFunctionType.Copy)
                    # stage this slice to DRAM for the per-query rescan gathers
                    eng = nc.gpsimd if s in (2, 5) else nc.sync
                    eng.dma_start(
                        out=affsDv[:, s * SLW:(s + 1) * SLW], in_=aff4[:])
                    # chunk-local pairwise-max tree: 4096 -> 64 chunk maxima
                    a3 = aff4[:].rearrange("p (g w) -> p g w", w=CW)
                    t1 = tre.tile([128, 2048], F16, tag="t1")
                    nc.vector.tensor_tensor(
                        out=t1[:].rearrange("p (g w) -> p g w", w=32),
                        in0=a3[:, :, 0:32], in1=a3[:, :, 32:64],
                        op=mybir.AluOpType.max)
                    t2 = tre.tile([128, 1024], F16, tag="t2")
                    nc.vector.tensor_tensor(
                        out=t2[:].rearrange("p (g w) -> p g w", w=16),
                        in0=t1[:].rearrange("p (g w) -> p g w", w=32)[:, :, 0:16],
                        in1=t1[:].rearrange("p (g w) -> p g w", w=32)[:, :, 16:32],
                        op=mybir.AluOpType.max)
                    t3 = tre.tile([128, 512], F16, tag="t3")
                    nc.vector.tensor_tensor(
                        out=t3[:].rearrange("p (g w) -> p g w", w=8),
                        in0=t2[:].rearrange("p (g w) -> p g w", w=16)[:, :, 0:8],
                        in1=t2[:].rearrange("p (g w) -> p g w", w=16)[:, :, 8:16],
                        op=mybir.AluOpType.max)
                    t4 = tre.tile([128, 256], F16, tag="t4")
                    nc.vector.tensor_tensor(
                        out=t4[:].rearrange("p (g w) -> p g w", w=4),
                        in0=t3[:].rearrange("p (g w) -> p g w", w=8)[:, :, 0:4],
                        in1=t3[:].rearrange("p (g w) -> p g w", w=8)[:, :, 4:8],
                        op=mybir.AluOpType.max)
                    t5 = tre.tile([128, 128], F16, tag="t5")
                    nc.vector.tensor_tensor(
                        out=t5[:].rearrange("p (g w) -> p g w", w=2),
                        in0=t4[:].rearrange("p (g w) -> p g w", w=4)[:, :, 0:2],
                        in1=t4[:].rearrange("p (g w) -> p g w", w=4)[:, :, 2:4],
                        op=mybir.AluOpType.max)
                    nc.vector.tensor_tensor(
                        out=cmax[:, s * 64:(s + 1) * 64],
                        in0=t5[:].rearrange("p (g w) -> p g w", w=2)[:, :, 0],
                        in1=t5[:].rearrange("p (g w) -> p g w", w=2)[:, :, 1],
                        op=mybir.AluOpType.max)

                # ---- select top-NSEL chunks per query (tie-free in f32) ----
                cmaxf = sel.tile([128, NCH], F32, tag="cmaxf")
                nc.vector.scalar_tensor_tensor(
                    out=cmaxf[:], in0=cmax[:], scalar=1.0, in1=ep512[:],
                    op0=mybir.AluOpType.mult, op1=mybir.AluOpType.add)
                cidu = sel.tile([128, NSEL], U32, tag="cidu")
                m8 = sel.tile([128, 8], F32, tag="m8")
                for r in range(NSEL // 8):
                    nc.vector.max(out=m8[:], in_=cmaxf[:])
                    nc.vector.max_index(
                        out=cidu[:, r * 8:(r + 1) * 8], in_max=m8[:],
                        in_values=cmaxf[:])
                    if r < NSEL // 8 - 1:
                        nc.vector.match_replace(
                            out=cmaxf[:], in_to_replace=m8[:],
                            in_values=cmaxf[:], imm_value=NEG)
                cidf = sel.tile([128, NSEL], F32, tag="cidf")
                nc.vector.tensor_copy(cidf[:], cidu[:])
                offf = sel.tile([128, NSEL], F32, tag="offf")
                nc.vector.tensor_scalar(
                    offf[:], cidf[:], pr512[:], None, op0=mybir.AluOpType.add)
                offu = sel.tile([128, NSEL], U32, tag="offu")
                nc.vector.tensor_copy(offu[:], offf[:])

                # ---- gather the selected chunks, rescan for top-8 each ----
                g24 = gat.tile([128, NSEL * CW], F16, tag="g24")
                nc.gpsimd.indirect_dma_start(
                    out=g24[:].rearrange("p (k w) -> p k w", w=CW),
                    out_offset=None, in_=affsD[:],
                    in_offset=bass.IndirectOffsetOnAxis(ap=offu[:], axis=0))
                # f32 + positional eps makes the rescan tie-free, so
                # max_index can't alias two tied elements to one position
                g24f = gat.tile([128, NSEL * CW], F32, tag="g24f")
                nc.vector.scalar_tensor_tensor(
                    out=g24f[:], in0=g24[:], scalar=1.0, in1=ep1536[:],
                    op0=mybir.AluOpType.mult, op1=mybir.AluOpType.add)
                cv8 = sel.tile([128, NCAND], F32, tag="cv8")
                pix = sel.tile([128, NCAND], U32, tag="pix")
                for j in range(NSEL):
                    nc.vector.max(
                        out=cv8[:, j * 8:(j + 1) * 8],
                        in_=g24f[:, j * CW:(j + 1) * CW])
                    nc.vector.max_index(
                        out=pix[:, j * 8:(j + 1) * 8],
                        in_max=cv8[:, j * 8:(j + 1) * 8],
                        in_values=g24f[:, j * CW:(j + 1) * CW])
                # decode element index: el = cid*64 + pix
                pixf = sel.tile([128, NCAND], F32, tag="pixf")
                nc.vector.tensor_copy(pixf[:], pix[:])
                elf = sel.tile([128, NCAND], F32, tag="elf")
                nc.vector.scalar_tensor_tensor(
                    out=elf[:].rearrange("p (k r) -> p k r", r=8),
                    in0=cidf[:].rearrange("p (k u) -> p k u", u=1)
                    .broadcast_to([128, NSEL, 8]),
                    scalar=float(CW),
                    in1=pixf[:].rearrange("p (k r) -> p k r", r=8),
                    op0=mybir.AluOpType.mult, op1=mybir.AluOpType.add)
                nc.sync.dma_start(
                    out=elD[:].rearrange("(p u) one -> p (u one)", p=128),
                    in_=elf[:])

                # ---- merge: exact top-20 of the 192 candidates ----
                cvf = sel.tile([128, NCAND], F32, tag="cvf")
                nc.vector.tensor_tensor(
                    out=cvf[:], in0=cv8[:], in1=ep192[:],
                    op=mybir.AluOpType.add)
                gvals = sel.tile([128, 24], F32, tag="gvals")
                gpos = sel.tile([128, 24], U32, tag="gpos")
                for r in range(3):
                    g8 = gvals[:, r * 8:(r + 1) * 8]
                    nc.vector.max(out=g8, in_=cvf[:])
                    nc.vector.max_index(
                        out=gpos[:, r * 8:(r + 1) * 8], in_max=g8,
                        in_values=cvf[:])
                    if r < 2:
                        nc.vector.match_replace(
                            out=cvf[:], in_to_replace=g8, in_values=cvf[:],
                            imm_value=NEG)
                # ---- exact fp32 rescore of the 24 candidates ----
                gposf = sel.tile([128, 24], F32, tag="gposf")
                nc.vector.tensor_copy(gposf[:], gpos[:])
                off2 = sel.tile([128, 24], F32, tag="off2")
                nc.vector.tensor_scalar(
                    off2[:], gposf[:], pr192[:], None, op0=mybir.AluOpType.add)
                offu2 = sel.tile([128, 24], U32, tag="offu2")
                nc.vector.tensor_copy(offu2[:], off2[:])
                el24 = sel.tile([128, 24], F32, tag="el24")
                nc.gpsimd.indirect_dma_start(
                    out=el24[:], out_offset=None, in_=elD[:],
                    in_offset=bass.IndirectOffsetOnAxis(ap=offu2[:], axis=0))
                el24u = sel.tile([128, 24], U32, tag="el24u")
                nc.vector.tensor_copy(el24u[:], el24[:])
                gmk = gbig.tile([128, 24 * (CK + 1)], F32, tag="gmk")
                nc.gpsimd.indirect_dma_start(
                    out=gmk[:].rearrange("p (k c) -> p k c", c=CK + 1),
                    out_offset=None, in_=mkT32[:],
                    in_offset=bass.IndirectOffsetOnAxis(ap=el24u[:], axis=0))
                # V rows for all 24 candidates, issued early so the gather
                # overlaps the rescore; non-top-20 slots get zero weight
                vTg = gbig.tile([128, 24 * 2 * CV], F16, tag="vTg")
                for hb in range(2):
                    nc.gpsimd.indirect_dma_start(
                        out=vTg[:, hb * 12 * 2 * CV:(hb + 1) * 12 * 2 * CV]
                        .rearrange("p (k c) -> p k c", c=2 * CV),
                        out_offset=None, in_=vTb[:],
                        in_offset=bass.IndirectOffsetOnAxis(
                            ap=el24u[:, hb * 12:(hb + 1) * 12], axis=0))
                nc.vector.tensor_tensor(
                    out=gmk[:].rearrange("p (k c) -> p k c", c=CK + 1),
                    in0=gmk[:].rearrange("p (k c) -> p k c", c=CK + 1),
                    in1=qrt[:].rearrange("p (u c) -> p u c", u=1)
                    .broadcast_to([128, 24, CK + 1]),
                    op=mybir.AluOpType.mult)
                av24 = sel.tile([128, 24], F32, tag="av24")
                nc.vector.tensor_reduce(
                    out=av24[:],
                    in_=gmk[:].rearrange("p (k c) -> p k c", c=CK + 1),
                    axis=mybir.AxisListType.X, op=mybir.AluOpType.add)
                nc.vector.tensor_tensor(
                    out=av24[:], in0=av24[:], in1=ep192[:, :24],
                    op=mybir.AluOpType.add)
                # ranked values (top-20 threshold), no positions needed
                av24c = sel.tile([128, 24], F32, tag="av24c")
                nc.vector.tensor_copy(av24c[:], av24[:])
                wvals = sel.tile([128, 24], F32, tag="wvals")
                for r in range(3):
                    w8 = wvals[:, r * 8:(r + 1) * 8]
                    nc.vector.max(out=w8, in_=av24c[:])
                    if r < 2:
                        nc.vector.match_replace(
                            out=av24c[:], in_to_replace=w8, in_values=av24c[:],
                            imm_value=NEG)

                # ---- masked softmax over all 24 slots (ranks >= 20 -> 0) ---
                mask = sel.tile([128, 24], F32, tag="mask")
                nc.vector.tensor_scalar(
                    mask[:], av24[:], wvals[:, 19:20], None,
                    op0=mybir.AluOpType.is_ge)
                negm40 = sel.tile([128, 1], F32, tag="negm40")
                nc.vector.tensor_scalar(
                    negm40[:], wvals[:, 0:1], -1.0, -40.0,
                    op0=mybir.AluOpType.mult, op1=mybir.AluOpType.add)
                # masked slots sit 40 below the kept ones -> exp ~ 0
                dms = sel.tile([128, 24], F32, tag="dms")
                nc.vector.scalar_tensor_tensor(
                    out=dms[:], in0=mask[:], scalar=40.0, in1=av24[:],
                    op0=mybir.AluOpType.mult, op1=mybir.AluOpType.add)
                wexp = sel.tile([128, 24], F32, tag="wexp")
                ssum = sel.tile([128, 1], F32, tag="ssum")
                nc.scalar.activation(
                    out=wexp[:], in_=dms[:],
                    func=mybir.ActivationFunctionType.Exp,
                    bias=negm40[:], scale=1.0, accum_out=ssum[:])
                rs = sel.tile([128, 1], F32, tag="rs")
                nc.vector.reciprocal(rs[:], ssum[:])
                wgt = sel.tile([128, 24], F32, tag="wgt")
                nc.vector.tensor_scalar(
                    wgt[:], wexp[:], rs[:], None, op0=mybir.AluOpType.mult)

                # ---- weighted readout over the 24 gathered V rows ----
                acc = gat.tile([128, 2 * CV], F32, tag="acc")
                nc.vector.memset(acc[:], 0.0)
                for k in range(24):
                    nc.vector.scalar_tensor_tensor(
                        out=acc[:], in0=vTg[:, k * 2 * CV:(k + 1) * 2 * CV],
                        scalar=wgt[:, k:k + 1], in1=acc[:],
                        op0=mybir.AluOpType.mult, op1=mybir.AluOpType.add)
                nc.sync.dma_start(
                    out=out[t * 128:(t + 1) * 128, :], in_=acc[:])
    nc.finalize()
    _prog_cache["p"] = nc
    return nc


def _host_inputs(qk, mem_k, mem_v1, mem_v2, top_k=TOPK):
    qk = np.asarray(qk, dtype=np.float32)
    mem_k = np.asarray(mem_k, dtype=np.float32)
    mem_v1 = np.asarray(mem_v1, dtype=np.float32)
    mem_v2 = np.asarray(mem_v2, dtype=np.float32)

    q2 = qk.reshape(CK, Q)
    a = np.sum(mem_k[0] * mem_k[0], axis=0, dtype=np.float32)      # [NE]
    na = -0.125 * a
    nh = na.astype(np.float16).astype(np.float32)
    nl = (na - nh).astype(np.float16)
    mkB = np.concatenate(
        [mem_k[0].astype(np.float16), nh.astype(np.float16)[None, :],
         nl[None, :]], axis=0)                                      # [66, NE]
    vTb = np.concatenate(
        [mem_v1[0].T, mem_v2[0].T], axis=1).astype(np.float16)      # [NE, 512]
    mkT32 = np.ascontiguousarray(np.concatenate(
        [mem_k[0].T, na[:, None]], axis=1, dtype=np.float32))      # [NE, 65]
    prow512 = (np.arange(128, dtype=np.float32) * NCH).reshape(128, 1)
    prow192 = (np.arange(128, dtype=np.float32) * NCAND).reshape(128, 1)
    eps512 = np.broadcast_to(
        np.arange(NCH, dtype=np.float32) * EPS, (128, NCH)).copy()
    eps192 = np.broadcast_to(
        np.arange(NCAND, dtype=np.float32) * EPS, (128, NCAND)).copy()
    eps1536 = np.broadcast_to(
        (np.arange(NSEL * CW, dtype=np.float32) % CW) * EPS,
        (128, NSEL * CW)).copy()

    in_maps = []
    for c in range(NC):
        sl = slice(c * Q_LOC, (c + 1) * Q_LOC)
        qs = 0.25 * q2[:, sl]
        qTb = np.concatenate(
            [qs.astype(np.float16), np.ones((2, Q_LOC), np.float16)],
            axis=0)                                                 # [66, 512]
        qrI = np.ascontiguousarray(np.concatenate(
            [qs.T, np.ones((Q_LOC, 1), np.float32)],
            axis=1, dtype=np.float32))                              # [512, 65]
        in_maps.append({
            "qTb": qTb, "mkB": mkB, "vTb": vTb,
            "mkT32": mkT32, "qrI": qrI,
            "prow512": prow512, "prow192": prow192,
            "eps512": eps512, "eps192": eps192, "eps1536": eps1536,
        })
    return in_maps


def _assemble_output(outs):
    full = np.concatenate(outs, axis=0)
    return np.ascontiguousarray(full.T).reshape(1, 2 * CV, H, W)


def kernel(qk, mem_k, mem_v1, mem_v2, top_k):
    assert int(top_k) == TOPK
    in_maps = _host_inputs(qk, mem_k, mem_v1, mem_v2)
    nc = _build_program()
    res = None
    for attempt in range(3):
        try:
            res = run_bass_kernel_spmd(nc, in_maps, core_ids=list(range(NC)))
            break
        except Exception:
            # transient device-unrecoverable states clear on the next attempt
            if attempt == 2:
                raise
            time.sleep(2.0)
    return _assemble_output([res.results[c]["out"] for c in range(NC)])


# revision 50
# speedup vs baseline: 1.1216x; 1.0115x over previous
import sys, time
sys.path.insert(0, "/opt/trn_rl_repo")
import numpy as np
from concourse import bass, bacc, mybir, tile
from concourse.bass_utils import run_bass_kernel_spmd

# Problem constants (nn_Memory_88656714925588)
B, CK, CV = 1, 64, 256
H, W, T = 64, 64, 8
NE = H * W * T            # 32768 memory elements
Q = H * W * 64 // 64      # 4096 queries
NC = 8                    # cores
Q_LOC = Q // NC           # 512 queries per core (query-sharded)
NQT = Q_LOC // 128        # 4 query tiles per core
TOPK = 20
CW = 64                   # chunk width for the screen
NCH = NE // CW            # 512 chunks per query row
NSEL = 24                 # chunks selected per query (>= 20 guarantees coverage)
NCAND = NSEL * 8          # 192 candidates after per-chunk top-8
NSLICE = 8                # 4096-column slices per tile
SLW = NE // NSLICE        # 4096
F32 = mybir.dt.float32
F16 = mybir.dt.float16
U32 = mybir.dt.uint32
NEG = -1e30
EPS = 2.0 ** -21

_prog_cache = {}


def _build_program():
    if "p" in _prog_cache:
        return _prog_cache["p"]
    nc = bacc.Bacc()
    qTb = nc.dram_tensor("qTb", [CK + 2, Q_LOC], F16, kind="ExternalInput")
    mkB = nc.dram_tensor("mkB", [CK + 2, NE], F16, kind="ExternalInput")
    eps1536 = nc.dram_tensor(
        "eps1536", [128, NSEL * CW], F32, kind="ExternalInput")
    vTb = nc.dram_tensor("vTb", [NE, 2 * CV], F16, kind="ExternalInput")
    mkT32 = nc.dram_tensor("mkT32", [NE, CK + 1], F32, kind="ExternalInput")
    qrI = nc.dram_tensor("qrI", [Q_LOC, CK + 1], F32, kind="ExternalInput")
    prow512 = nc.dram_tensor("prow512", [128, 1], F32, kind="ExternalInput")
    prow192 = nc.dram_tensor("prow192", [128, 1], F32, kind="ExternalInput")
    eps512 = nc.dram_tensor("eps512", [128, NCH], F32, kind="ExternalInput")
    eps192 = nc.dram_tensor("eps192", [128, NCAND], F32, kind="ExternalInput")
    out = nc.dram_tensor("out", [Q_LOC, 2 * CV], F32, kind="ExternalOutput")

    with tile.TileContext(nc) as tc:
        with tc.tile_pool(name="cst", bufs=1) as cst, \
             tc.tile_pool(name="aff", bufs=3) as affp, \
             tc.tile_pool(name="tree", bufs=2) as tre, \
             tc.tile_pool(name="sel", bufs=2) as sel, \
             tc.tile_pool(name="gat", bufs=2) as gat, \
             tc.tile_pool(name="gbig", bufs=1) as gbig, \
             tc.tile_pool(name="psum", bufs=2, space="PSUM") as psum, \
             tc.tile_pool(name="dram", bufs=2, space="DRAM") as dram:

            qt = cst.tile([CK + 2, Q_LOC], F16)
            mkt = cst.tile([CK + 2, NE], F16)
            # small inputs first so the first matmul isn't queued behind
            # the big mk transfers
            nc.sync.dma_start(out=qt[:], in_=qTb[:])
            ep1536 = cst.tile([128, NSEL * CW], F32)
            nc.gpsimd.dma_start(out=ep1536[:], in_=eps1536[:])
            pr512 = cst.tile([128, 1], F32)
            nc.sync.dma_start(out=pr512[:], in_=prow512[:])
            pr192 = cst.tile([128, 1], F32)
            nc.sync.dma_start(out=pr192[:], in_=prow192[:])
            ep512 = cst.tile([128, NCH], F32)
            nc.gpsimd.dma_start(out=ep512[:], in_=eps512[:])
            ep192 = cst.tile([128, NCAND], F32)
            nc.gpsimd.dma_start(out=ep192[:], in_=eps192[:])
            # chunked mk load, split across queues; tiny first chunk so the
            # first matmul can start almost immediately
            bounds = [0, 512, 1024, 2048, 4096] + \
                [SLW * i for i in range(2, NSLICE + 1)]
            for ci in range(len(bounds) - 1):
                eng = nc.sync if ci % 2 == 0 else nc.gpsimd
                eng.dma_start(
                    out=mkt[:, bounds[ci]:bounds[ci + 1]],
                    in_=mkB[:, bounds[ci]:bounds[ci + 1]])

            for t in range(NQT):
                qrt = sel.tile([128, CK + 1], F32, tag="qrt")
                nc.sync.dma_start(
                    out=qrt[:], in_=qrI[t * 128:(t + 1) * 128, :])
                affsD = dram.tile([128 * NCH, CW], F16, tag="affsD")
                affsDv = affsD[:].rearrange("(p c) w -> p (c w)", p=128)
                elD = dram.tile([128 * NCAND, 1], F32, tag="elD")
                cmax = tre.tile([128, NCH], F16, tag="cmax")

                for s in range(NSLICE):
                    aff4 = affp.tile([128, SLW], F16, tag="aff4")
                    for h in range(2):
                        ph = psum.tile([128, 2048], F32, tag="ph")
                        for c in range(4):
                            col = s * SLW + h * 2048 + c * 512
                            nc.tensor.matmul(
                                out=ph[:, c * 512:(c + 1) * 512],
                                lhsT=qt[:, t * 128:(t + 1) * 128],
                                rhs=mkt[:, col:col + 512],
                                start=True, stop=True)
                        nc.scalar.activation(
                            out=aff4[:, h * 2048:(h + 1) * 2048], in_=ph[:],
                            func=mybir.ActivationFunctionType.Copy)
                    # stage this slice to DRAM for the per-query rescan gathers
                    eng = nc.gpsimd if s in (2, 5) else nc.sync
                    eng.dma_start(
                        out=affsDv[:, s * SLW:(s + 1) * SLW], in_=aff4[:])
                    # chunk-local pairwise-max tree: 4096 -> 64 chunk maxima
                    a3 = aff4[:].rearrange("p (g w) -> p g w", w=CW)
                    t1 = tre.tile([128, 2048], F16, tag="t1")
                    nc.vector.tensor_tensor(
                        out=t1[:].rearrange("p (g w) -> p g w", w=32),
                        in0=a3[:, :, 0:32], in1=a3[:, :, 32:64],
                        op=mybir.AluOpType.max)
                    t2 = tre.tile([128, 1024], F16, tag="t2")
                    nc.vector.tensor_tensor(
                        out=t2[:].rearrange("p (g w) -> p g w", w=16),
                        in0=t1[:].rearrange("p (g w) -> p g w", w=32)[:, :, 0:16],
                        in1=t1[:].rearrange("p (g w) -> p g w", w=32)[:, :, 16:32],
                        op=mybir.AluOpType.max)
                    t3 = tre.tile([128, 512], F16, tag="t3")
                    nc.vector.tensor_tensor(
                        out=t3[:].rearrange("p (g w) -> p g w", w=8),
                        in0=t2[:].rearrange("p (g w) -> p g w", w=16)[:, :, 0:8],
                        in1=t2[:].rearrange("p (g w) -> p g w", w=16)[:, :, 8:16],
                        op=mybir.AluOpType.max)
                    t4 = tre.tile([128, 256], F16, tag="t4")
                    nc.vector.tensor_tensor(
                        out=t4[:].rearrange("p (g w) -> p g w", w=4),
                        in0=t3[:].rearrange("p (g w) -> p g w", w=8)[:, :, 0:4],
                        in1=t3[:].rearrange("p (g w) -> p g w", w=8)[:, :, 4:8],
                        op=mybir.AluOpType.max)
                    t5 = tre.tile([128, 128], F16, tag="t5")
                    nc.vector.tensor_tensor(
                        out=t5[:].rearrange("p (g w) -> p g w", w=2),
                        in0=t4[:].rearrange("p (g w) -> p g w", w=4)[:, :, 0:2],
                        in1=t4[:].rearrange("p (g w) -> p g w", w=4)[:, :, 2:4],
                        op=mybir.AluOpType.max)
                    nc.vector.tensor_tensor(
                        out=cmax[:, s * 64:(s + 1) * 64],
                        in0=t5[:].rearrange("p (g w) -> p g w", w=2)[:, :, 0],
                        in1=t5[:].rearrange("p (g w) -> p g w", w=2)[:, :, 1],
                        op=mybir.AluOpType.max)

                # ---- select top-NSEL chunks per query (tie-free in f32) ----
                cmaxf = sel.tile([128, NCH], F32, tag="cmaxf")
                nc.vector.scalar_tensor_tensor(
                    out=cmaxf[:], in0=cmax[:], scalar=1.0, in1=ep512[:],
                    op0=mybir.AluOpType.mult, op1=mybir.AluOpType.add)
                cidu = sel.tile([128, NSEL], U32, tag="cidu")
                m8 = sel.tile([128, 8], F32, tag="m8")
                for r in range(NSEL // 8):
                    nc.vector.max(out=m8[:], in_=cmaxf[:])
                    nc.vector.max_index(
                        out=cidu[:, r * 8:(r + 1) * 8], in_max=m8[:],
                        in_values=cmaxf[:])
                    if r < NSEL // 8 - 1:
                        nc.vector.match_replace(
                            out=cmaxf[:], in_to_replace=m8[:],
                            in_values=cmaxf[:], imm_value=NEG)
                cidf = sel.tile([128, NSEL], F32, tag="cidf")
                nc.vector.tensor_copy(cidf[:], cidu[:])
                offf = sel.tile([128, NSEL], F32, tag="offf")
                nc.vector.tensor_scalar(
                    offf[:], cidf[:], pr512[:], None, op0=mybir.AluOpType.add)
                offu = sel.tile([128, NSEL], U32, tag="offu")
                nc.vector.tensor_copy(offu[:], offf[:])

                # ---- gather the selected chunks, rescan for top-8 each ----
                g24 = gat.tile([128, NSEL * CW], F16, tag="g24")
                nc.gpsimd.indirect_dma_start(
                    out=g24[:].rearrange("p (k w) -> p k w", w=CW),
                    out_offset=None, in_=affsD[:],
                    in_offset=bass.IndirectOffsetOnAxis(ap=offu[:], axis=0))
                # multiplicative positional eps: v' = v*(1 + pos*2^-19).
                # v is on the f16 grid, so round-to-f16 recovers v and the
                # ratio recovers pos -- no max_index pass needed.
                g24f = gat.tile([128, NSEL * CW], F32, tag="g24f")
                nc.vector.tensor_tensor(
                    out=g24f[:], in0=g24[:], in1=ep1536[:],
                    op=mybir.AluOpType.mult)
                cv8 = sel.tile([128, NCAND], F32, tag="cv8")
                for j in range(NSEL):
                    nc.vector.max(
                        out=cv8[:, j * 8:(j + 1) * 8],
                        in_=g24f[:, j * CW:(j + 1) * CW])
                vq16 = sel.tile([128, NCAND], F16, tag="vq16")
                nc.vector.tensor_copy(vq16[:], cv8[:])
                # denominator guard: +-1e-30 by sign so v=0 can't divide by 0
                sg = sel.tile([128, NCAND], F32, tag="sg")
                nc.vector.tensor_scalar(
                    sg[:], vq16[:], 0.0, None, op0=mybir.AluOpType.is_ge)
                nc.vector.tensor_scalar(
                    sg[:], sg[:], 2e-30, -1e-30,
                    op0=mybir.AluOpType.mult, op1=mybir.AluOpType.add)
                vq2 = sel.tile([128, NCAND], F32, tag="vq2")
                nc.vector.tensor_tensor(
                    out=vq2[:], in0=vq16[:], in1=sg[:],
                    op=mybir.AluOpType.add)
                pf = sel.tile([128, NCAND], F32, tag="pf")
                nc.vector.tensor_tensor(
                    out=pf[:], in0=cv8[:], in1=vq2[:],
                    op=mybir.AluOpType.divide)
                nc.vector.tensor_scalar(
                    pf[:], pf[:], float(2 ** 19), -float(2 ** 19) + 0.5,
                    op0=mybir.AluOpType.mult, op1=mybir.AluOpType.add)
                nc.vector.tensor_scalar(
                    pf[:], pf[:], 0.0, None, op0=mybir.AluOpType.max)
                nc.vector.tensor_scalar(
                    pf[:], pf[:], 63.9, None, op0=mybir.AluOpType.min)
                elf = sel.tile([128, NCAND], F32, tag="elf")
                nc.vector.scalar_tensor_tensor(
                    out=elf[:].rearrange("p (k r) -> p k r", r=8),
                    in0=cidf[:].rearrange("p (k u) -> p k u", u=1)
                    .broadcast_to([128, NSEL, 8]),
                    scalar=float(CW),
                    in1=pf[:].rearrange("p (k r) -> p k r", r=8),
                    op0=mybir.AluOpType.mult, op1=mybir.AluOpType.add)
                nc.sync.dma_start(
                    out=elD[:].rearrange("(p u) one -> p (u one)", p=128),
                    in_=elf[:])

                # ---- merge: exact top-20 of the 192 candidates ----
                cvf = sel.tile([128, NCAND], F32, tag="cvf")
                nc.vector.tensor_tensor(
                    out=cvf[:], in0=cv8[:], in1=ep192[:],
                    op=mybir.AluOpType.add)
                gvals = sel.tile([128, 24], F32, tag="gvals")
                gpos = sel.tile([128, 24], U32, tag="gpos")
                for r in range(3):
                    g8 = gvals[:, r * 8:(r + 1) * 8]
                    nc.vector.max(out=g8, in_=cvf[:])
                    nc.vector.max_index(
                        out=gpos[:, r * 8:(r + 1) * 8], in_max=g8,
                        in_values=cvf[:])
                    if r < 2:
                        nc.vector.match_replace(
                            out=cvf[:], in_to_replace=g8, in_values=cvf[:],
                            imm_value=NEG)
                # ---- exact fp32 rescore of the 24 candidates ----
                gposf = sel.tile([128, 24], F32, tag="gposf")
                nc.vector.tensor_copy(gposf[:], gpos[:])
                off2 = sel.tile([128, 24], F32, tag="off2")
                nc.vector.tensor_scalar(
                    off2[:], gposf[:], pr192[:], None, op0=mybir.AluOpType.add)
                offu2 = sel.tile([128, 24], U32, tag="offu2")
                nc.vector.tensor_copy(offu2[:], off2[:])
                el24 = sel.tile([128, 24], F32, tag="el24")
                nc.gpsimd.indirect_dma_start(
                    out=el24[:], out_offset=None, in_=elD[:],
                    in_offset=bass.IndirectOffsetOnAxis(ap=offu2[:], axis=0))
                el24u = sel.tile([128, 24], U32, tag="el24u")
                nc.vector.tensor_copy(el24u[:], el24[:])
                gmk = gbig.tile([128, 24 * (CK + 1)], F32, tag="gmk")
                nc.gpsimd.indirect_dma_start(
                    out=gmk[:].rearrange("p (k c) -> p k c", c=CK + 1),
                    out_offset=None, in_=mkT32[:],
                    in_offset=bass.IndirectOffsetOnAxis(ap=el24u[:], axis=0))
                # V rows for all 24 candidates, issued early so the gather
                # overlaps the rescore; non-top-20 slots get zero weight
                vTg = gbig.tile([128, 24 * 2 * CV], F16, tag="vTg")
                for hb in range(2):
                    nc.gpsimd.indirect_dma_start(
                        out=vTg[:, hb * 12 * 2 * CV:(hb + 1) * 12 * 2 * CV]
                        .rearrange("p (k c) -> p k c", c=2 * CV),
                        out_offset=None, in_=vTb[:],
                        in_offset=bass.IndirectOffsetOnAxis(
                            ap=el24u[:, hb * 12:(hb + 1) * 12], axis=0))
                nc.vector.tensor_tensor(
                    out=gmk[:].rearrange("p (k c) -> p k c", c=CK + 1),
                    in0=gmk[:].rearrange("p (k c) -> p k c", c=CK + 1),
                    in1=qrt[:].rearrange("p (u c) -> p u c", u=1)
                    .broadcast_to([128, 24, CK + 1]),
                    op=mybir.AluOpType.mult)
                av24 = sel.tile([128, 24], F32, tag="av24")
                nc.vector.tensor_reduce(
                    out=av24[:],
                    in_=gmk[:].rearrange("p (k c) -> p k c", c=CK + 1),
                    axis=mybir.AxisListType.X, op=mybir.AluOpType.add)
                nc.vector.tensor_tensor(
                    out=av24[:], in0=av24[:], in1=ep192[:, :24],
                    op=mybir.AluOpType.add)
                # ranked values (top-20 threshold), no positions needed
                av24c = sel.tile([128, 24], F32, tag="av24c")
                nc.vector.tensor_copy(av24c[:], av24[:])
                wvals = sel.tile([128, 24], F32, tag="wvals")
                for r in range(3):
                    w8 = wvals[:, r * 8:(r + 1) * 8]
                    nc.vector.max(out=w8, in_=av24c[:])
                    if r < 2:
                        nc.vector.match_replace(
                            out=av24c[:], in_to_replace=w8, in_values=av24c[:],
                            imm_value=NEG)

                # ---- masked softmax over all 24 slots (ranks >= 20 -> 0) ---
                mask = sel.tile([128, 24], F32, tag="mask")
                nc.vector.tensor_scalar(
                    mask[:], av24[:], wvals[:, 19:20], None,
                    op0=mybir.AluOpType.is_ge)
                negm40 = sel.tile([128, 1], F32, tag="negm40")
                nc.vector.tensor_scalar(
                    negm40[:], wvals[:, 0:1], -1.0, -40.0,
                    op0=mybir.AluOpType.mult, op1=mybir.AluOpType.add)
                # masked slots sit 40 below the kept ones -> exp ~ 0
                dms = sel.tile([128, 24], F32, tag="dms")
                nc.vector.scalar_tensor_tensor(
                    out=dms[:], in0=mask[:], scalar=40.0, in1=av24[:],
                    op0=mybir.AluOpType.mult, op1=mybir.AluOpType.add)
                wexp = sel.tile([128, 24], F32, tag="wexp")
                ssum = sel.tile([128, 1], F32, tag="ssum")
                nc.scalar.activation(
                    out=wexp[:], in_=dms[:],
                    func=mybir.ActivationFunctionType.Exp,
                    bias=negm40[:], scale=1.0, accum_out=ssum[:])
                rs = sel.tile([128, 1], F32, tag="rs")
                nc.vector.reciprocal(rs[:], ssum[:])
                wgt = sel.tile([128, 24], F32, tag="wgt")
                nc.vector.tensor_scalar(
                    wgt[:], wexp[:], rs[:], None, op0=mybir.AluOpType.mult)

                # ---- weighted readout over the 24 gathered V rows ----
                acc = gat.tile([128, 2 * CV], F32, tag="acc")
                nc.vector.memset(acc[:], 0.0)
                for k in range(24):
                    nc.vector.scalar_tensor_tensor(
                        out=acc[:], in0=vTg[:, k * 2 * CV:(k + 1) * 2 * CV],
                        scalar=wgt[:, k:k + 1], in1=acc[:],
                        op0=mybir.AluOpType.mult, op1=mybir.AluOpType.add)
                nc.sync.dma_start(
                    out=out[t * 128:(t + 1) * 128, :], in_=acc[:])
    nc.finalize()
    _prog_cache["p"] = nc
    return nc


def _host_inputs(qk, mem_k, mem_v1, mem_v2, top_k=TOPK):
    qk = np.asarray(qk, dtype=np.float32)
    mem_k = np.asarray(mem_k, dtype=np.float32)
    mem_v1 = np.asarray(mem_v1, dtype=np.float32)
    mem_v2 = np.asarray(mem_v2, dtype=np.float32)

    q2 = qk.reshape(CK, Q)
    a = np.sum(mem_k[0] * mem_k[0], axis=0, dtype=np.float32)      # [NE]
    na = -0.125 * a
    nh = na.astype(np.float16).astype(np.float32)
    nl = (na - nh).astype(np.float16)
    mkB = np.concatenate(
        [mem_k[0].astype(np.float16), nh.astype(np.float16)[None, :],
         nl[None, :]], axis=0)                                      # [66, NE]
    vTb = np.concatenate(
        [mem_v1[0].T, mem_v2[0].T], axis=1).astype(np.float16)      # [NE, 512]
    mkT32 = np.ascontiguousarray(np.concatenate(
        [mem_k[0].T, na[:, None]], axis=1, dtype=np.float32))      # [NE, 65]
    prow512 = (np.arange(128, dtype=np.float32) * NCH).reshape(128, 1)
    prow192 = (np.arange(128, dtype=np.float32) * NCAND).reshape(128, 1)
    eps512 = np.broadcast_to(
        np.arange(NCH, dtype=np.float32) * EPS, (128, NCH)).copy()
    eps192 = np.broadcast_to(
        np.arange(NCAND, dtype=np.float32) * EPS, (128, NCAND)).copy()
    eps1536 = np.broadcast_to(
        1.0 + (np.arange(NSEL * CW, dtype=np.float32) % CW) * 2.0 ** -19,
        (128, NSEL * CW)).astype(np.float32).copy()

    in_maps = []
    for c in range(NC):
        sl = slice(c * Q_LOC, (c + 1) * Q_LOC)
        qs = 0.25 * q2[:, sl]
        qTb = np.concatenate(
            [qs.astype(np.float16), np.ones((2, Q_LOC), np.float16)],
            axis=0)                                                 # [66, 512]
        qrI = np.ascontiguousarray(np.concatenate(
            [qs.T, np.ones((Q_LOC, 1), np.float32)],
            axis=1, dtype=np.float32))                              # [512, 65]
        in_maps.append({
            "qTb": qTb, "mkB": mkB, "vTb": vTb,
            "mkT32": mkT32, "qrI": qrI,
            "prow512": prow512, "prow192": prow192,
            "eps512": eps512, "eps192": eps192, "eps1536": eps1536,
        })
    return in_maps


def _assemble_output(outs):
    full = np.concatenate(outs, axis=0)
    return np.ascontiguousarray(full.T).reshape(1, 2 * CV, H, W)


def kernel(qk, mem_k, mem_v1, mem_v2, top_k):
    assert int(top_k) == TOPK
    in_maps = _host_inputs(qk, mem_k, mem_v1, mem_v2)
    nc = _build_program()
    res = None
    for attempt in range(3):
        try:
            res = run_bass_kernel_spmd(nc, in_maps, core_ids=list(range(NC)))
            break
        except Exception:
            # transient device-unrecoverable states clear on the next attempt
            if attempt == 2:
                raise
            time.sleep(2.0)
    return _assemble_output([res.results[c]["out"] for c in range(NC)])


# revision 55
# speedup vs baseline: 1.1702x; 1.0433x over previous
import sys, time
sys.path.insert(0, "/opt/trn_rl_repo")
import numpy as np
from concourse import bass, bacc, mybir, tile
from concourse.bass_utils import run_bass_kernel_spmd

# Problem constants (nn_Memory_88656714925588)
B, CK, CV = 1, 64, 256
H, W, T = 64, 64, 8
NE = H * W * T            # 32768 memory elements
Q = H * W * 64 // 64      # 4096 queries
NC = 8                    # cores
Q_LOC = Q // NC           # 512 queries per core (query-sharded)
NQT = Q_LOC // 128        # 4 query tiles per core
TOPK = 20
CW = 64                   # chunk width for the screen
NCH = NE // CW            # 512 chunks per query row
NSEL = 24                 # chunks selected per query (>= 20 guarantees coverage)
NPR = NSEL // 2           # rescan processes chunk pairs
NCAND = NPR * 8           # 96 candidates after per-pair top-8
NSLICE = 8                # 4096-column slices per tile
SLW = NE // NSLICE        # 4096
F32 = mybir.dt.float32
F16 = mybir.dt.float16
U32 = mybir.dt.uint32
NEG = -1e30
EPS = 2.0 ** -21

_prog_cache = {}


def _build_program():
    if "p" in _prog_cache:
        return _prog_cache["p"]
    nc = bacc.Bacc()
    qTb = nc.dram_tensor("qTb", [CK + 2, Q_LOC], F16, kind="ExternalInput")
    mkB = nc.dram_tensor("mkB", [CK + 2, NE], F16, kind="ExternalInput")
    eps1536 = nc.dram_tensor(
        "eps1536", [128, NSEL * CW], F32, kind="ExternalInput")
    vTb = nc.dram_tensor("vTb", [NE, 2 * CV], F16, kind="ExternalInput")
    mkT32 = nc.dram_tensor("mkT32", [NE, CK + 1], F32, kind="ExternalInput")
    qrI = nc.dram_tensor("qrI", [Q_LOC, CK + 1], F32, kind="ExternalInput")
    prow512 = nc.dram_tensor("prow512", [128, 1], F32, kind="ExternalInput")
    prow192 = nc.dram_tensor("prow192", [128, 1], F32, kind="ExternalInput")
    eps512 = nc.dram_tensor("eps512", [128, NCH], F32, kind="ExternalInput")
    eps192 = nc.dram_tensor("eps192", [128, NCAND], F32, kind="ExternalInput")
    out = nc.dram_tensor("out", [Q_LOC, 2 * CV], F32, kind="ExternalOutput")

    with tile.TileContext(nc) as tc:
        with tc.tile_pool(name="cst", bufs=1) as cst, \
             tc.tile_pool(name="aff", bufs=3) as affp, \
             tc.tile_pool(name="tree", bufs=2) as tre, \
             tc.tile_pool(name="sel", bufs=2) as sel, \
             tc.tile_pool(name="gat", bufs=2) as gat, \
             tc.tile_pool(name="gbig", bufs=1) as gbig, \
             tc.tile_pool(name="psum", bufs=2, space="PSUM") as psum, \
             tc.tile_pool(name="dram", bufs=2, space="DRAM") as dram:

            qt = cst.tile([CK + 2, Q_LOC], F16)
            mkt = cst.tile([CK + 2, NE], F16)
            # small inputs first so the first matmul isn't queued behind
            # the big mk transfers
            nc.sync.dma_start(out=qt[:], in_=qTb[:])
            ep1536 = cst.tile([128, NSEL * CW], F32)
            nc.gpsimd.dma_start(out=ep1536[:], in_=eps1536[:])
            pr512 = cst.tile([128, 1], F32)
            nc.sync.dma_start(out=pr512[:], in_=prow512[:])
            pr192 = cst.tile([128, 1], F32)
            nc.sync.dma_start(out=pr192[:], in_=prow192[:])
            ep512 = cst.tile([128, NCH], F32)
            nc.gpsimd.dma_start(out=ep512[:], in_=eps512[:])
            ep192 = cst.tile([128, NCAND], F32)
            nc.gpsimd.dma_start(out=ep192[:], in_=eps192[:])
            # chunked mk load, split across queues; tiny first chunk so the
            # first matmul can start almost immediately
            bounds = [0, 512, 1024, 2048, 4096] + \
                [SLW * i for i in range(2, NSLICE + 1)]
            for ci in range(len(bounds) - 1):
                eng = nc.sync if ci % 2 == 0 else nc.gpsimd
                eng.dma_start(
                    out=mkt[:, bounds[ci]:bounds[ci + 1]],
                    in_=mkB[:, bounds[ci]:bounds[ci + 1]])

            for t in range(NQT):
                qrt = sel.tile([128, CK + 1], F32, tag="qrt")
                nc.sync.dma_start(
                    out=qrt[:], in_=qrI[t * 128:(t + 1) * 128, :])
                affsD = dram.tile([128 * NCH, CW], F16, tag="affsD")
                affsDv = affsD[:].rearrange("(p c) w -> p (c w)", p=128)
                elD = dram.tile([128 * NCAND, 1], F32, tag="elD")
                cmax = tre.tile([128, NCH], F16, tag="cmax")

                for s in range(NSLICE):
                    aff4 = affp.tile([128, SLW], F16, tag="aff4")
                    for h in range(2):
                        ph = psum.tile([128, 2048], F32, tag="ph")
                        for c in range(4):
                            col = s * SLW + h * 2048 + c * 512
                            nc.tensor.matmul(
                                out=ph[:, c * 512:(c + 1) * 512],
                                lhsT=qt[:, t * 128:(t + 1) * 128],
                                rhs=mkt[:, col:col + 512],
                                start=True, stop=True)
                        nc.scalar.activation(
                            out=aff4[:, h * 2048:(h + 1) * 2048], in_=ph[:],
                            func=mybir.ActivationFunctionType.Copy)
                    # stage this slice to DRAM for the per-query rescan gathers
                    eng = nc.gpsimd if s in (2, 5) else nc.sync
                    eng.dma_start(
                        out=affsDv[:, s * SLW:(s + 1) * SLW], in_=aff4[:])
                    # chunk-local pairwise-max tree: 4096 -> 64 chunk maxima
                    a3 = aff4[:].rearrange("p (g w) -> p g w", w=CW)
                    t1 = tre.tile([128, 2048], F16, tag="t1")
                    nc.vector.tensor_tensor(
                        out=t1[:].rearrange("p (g w) -> p g w", w=32),
                        in0=a3[:, :, 0:32], in1=a3[:, :, 32:64],
                        op=mybir.AluOpType.max)
                    t2 = tre.tile([128, 1024], F16, tag="t2")
                    nc.vector.tensor_tensor(
                        out=t2[:].rearrange("p (g w) -> p g w", w=16),
                        in0=t1[:].rearrange("p (g w) -> p g w", w=32)[:, :, 0:16],
                        in1=t1[:].rearrange("p (g w) -> p g w", w=32)[:, :, 16:32],
                        op=mybir.AluOpType.max)
                    t3 = tre.tile([128, 512], F16, tag="t3")
                    nc.vector.tensor_tensor(
                        out=t3[:].rearrange("p (g w) -> p g w", w=8),
                        in0=t2[:].rearrange("p (g w) -> p g w", w=16)[:, :, 0:8],
                        in1=t2[:].rearrange("p (g w) -> p g w", w=16)[:, :, 8:16],
                        op=mybir.AluOpType.max)
                    t4 = tre.tile([128, 256], F16, tag="t4")
                    nc.vector.tensor_tensor(
                        out=t4[:].rearrange("p (g w) -> p g w", w=4),
                        in0=t3[:].rearrange("p (g w) -> p g w", w=8)[:, :, 0:4],
                        in1=t3[:].rearrange("p (g w) -> p g w", w=8)[:, :, 4:8],
                        op=mybir.AluOpType.max)
                    t5 = tre.tile([128, 128], F16, tag="t5")
                    nc.vector.tensor_tensor(
                        out=t5[:].rearrange("p (g w) -> p g w", w=2),
                        in0=t4[:].rearrange("p (g w) -> p g w", w=4)[:, :, 0:2],
                        in1=t4[:].rearrange("p (g w) -> p g w", w=4)[:, :, 2:4],
                        op=mybir.AluOpType.max)
                    nc.vector.tensor_tensor(
                        out=cmax[:, s * 64:(s + 1) * 64],
                        in0=t5[:].rearrange("p (g w) -> p g w", w=2)[:, :, 0],
                        in1=t5[:].rearrange("p (g w) -> p g w", w=2)[:, :, 1],
                        op=mybir.AluOpType.max)

                # ---- select top-NSEL chunks per query (tie-free in f32) ----
                cmaxf = sel.tile([128, NCH], F32, tag="cmaxf")
                nc.vector.scalar_tensor_tensor(
                    out=cmaxf[:], in0=cmax[:], scalar=1.0, in1=ep512[:],
                    op0=mybir.AluOpType.mult, op1=mybir.AluOpType.add)
                cidu = sel.tile([128, NSEL], U32, tag="cidu")
                m8 = sel.tile([128, 8], F32, tag="m8")
                for r in range(NSEL // 8):
                    nc.vector.max(out=m8[:], in_=cmaxf[:])
                    nc.vector.max_index(
                        out=cidu[:, r * 8:(r + 1) * 8], in_max=m8[:],
                        in_values=cmaxf[:])
                    if r < NSEL // 8 - 1:
                        nc.vector.match_replace(
                            out=cmaxf[:], in_to_replace=m8[:],
                            in_values=cmaxf[:], imm_value=NEG)
                cidf = sel.tile([128, NSEL], F32, tag="cidf")
                nc.vector.tensor_copy(cidf[:], cidu[:])
                offf = sel.tile([128, NSEL], F32, tag="offf")
                nc.vector.tensor_scalar(
                    offf[:], cidf[:], pr512[:], None, op0=mybir.AluOpType.add)
                offu = sel.tile([128, NSEL], U32, tag="offu")
                nc.vector.tensor_copy(offu[:], offf[:])

                # ---- gather the selected chunks, rescan for top-8 each ----
                g24 = gat.tile([128, NSEL * CW], F16, tag="g24")
                nc.gpsimd.indirect_dma_start(
                    out=g24[:].rearrange("p (k w) -> p k w", w=CW),
                    out_offset=None, in_=affsD[:],
                    in_offset=bass.IndirectOffsetOnAxis(ap=offu[:], axis=0))
                # multiplicative positional eps: v' = v*(1 + pos*2^-19).
                # v is on the f16 grid, so round-to-f16 recovers v and the
                # ratio recovers pos -- no max_index pass needed.
                g24f = gat.tile([128, NSEL * CW], F32, tag="g24f")
                nc.vector.tensor_tensor(
                    out=g24f[:], in0=g24[:], in1=ep1536[:],
                    op=mybir.AluOpType.mult)
                cv8 = sel.tile([128, NCAND], F32, tag="cv8")
                for j in range(NPR):
                    nc.vector.max(
                        out=cv8[:, j * 8:(j + 1) * 8],
                        in_=g24f[:, j * 2 * CW:(j + 1) * 2 * CW])
                vq16 = sel.tile([128, NCAND], F16, tag="vq16")
                nc.vector.tensor_copy(vq16[:], cv8[:])
                # denominator guard: +-1e-30 by sign so v=0 can't divide by 0
                sg = sel.tile([128, NCAND], F32, tag="sg")
                nc.vector.tensor_scalar(
                    sg[:], vq16[:], 0.0, None, op0=mybir.AluOpType.is_ge)
                nc.vector.tensor_scalar(
                    sg[:], sg[:], 2e-30, -1e-30,
                    op0=mybir.AluOpType.mult, op1=mybir.AluOpType.add)
                vq2 = sel.tile([128, NCAND], F32, tag="vq2")
                nc.vector.tensor_tensor(
                    out=vq2[:], in0=vq16[:], in1=sg[:],
                    op=mybir.AluOpType.add)
                pf = sel.tile([128, NCAND], F32, tag="pf")
                nc.vector.tensor_tensor(
                    out=pf[:], in0=cv8[:], in1=vq2[:],
                    op=mybir.AluOpType.divide)
                nc.vector.tensor_scalar(
                    pf[:], pf[:], float(2 ** 21), -float(2 ** 21) + 0.5,
                    op0=mybir.AluOpType.mult, op1=mybir.AluOpType.add)
                nc.vector.tensor_scalar(
                    pf[:], pf[:], 0.0, None, op0=mybir.AluOpType.max)
                nc.vector.tensor_scalar(
                    pf[:], pf[:], 127.9, None, op0=mybir.AluOpType.min)
                # pair decode: el = cid[2j]*64 + pf, or the odd chunk's base
                # when pf falls in the second half of the gathered pair
                cid3 = cidf[:].rearrange("p (j two) -> p j two", two=2)
                cdm = sel.tile([128, NPR], F32, tag="cdm")
                nc.vector.tensor_tensor(
                    out=cdm[:], in0=cid3[:, :, 1], in1=cid3[:, :, 0],
                    op=mybir.AluOpType.subtract)
                nc.vector.tensor_scalar(
                    cdm[:], cdm[:], float(CW), -float(CW),
                    op0=mybir.AluOpType.mult, op1=mybir.AluOpType.add)
                ge = sel.tile([128, NCAND], F32, tag="ge")
                nc.vector.tensor_scalar(
                    ge[:], pf[:], float(CW), None, op0=mybir.AluOpType.is_ge)
                nc.vector.tensor_tensor(
                    out=ge[:].rearrange("p (j r) -> p j r", r=8),
                    in0=ge[:].rearrange("p (j r) -> p j r", r=8),
                    in1=cdm[:].rearrange("p (j u) -> p j u", u=1)
                    .broadcast_to([128, NPR, 8]),
                    op=mybir.AluOpType.mult)
                elf = sel.tile([128, NCAND], F32, tag="elf")
                nc.vector.scalar_tensor_tensor(
                    out=elf[:].rearrange("p (j r) -> p j r", r=8),
                    in0=cid3[:, :, 0:1].broadcast_to([128, NPR, 8]),
                    scalar=float(CW),
                    in1=pf[:].rearrange("p (j r) -> p j r", r=8),
                    op0=mybir.AluOpType.mult, op1=mybir.AluOpType.add)
                nc.vector.tensor_tensor(
                    out=elf[:], in0=elf[:], in1=ge[:],
                    op=mybir.AluOpType.add)
                nc.sync.dma_start(
                    out=elD[:].rearrange("(p u) one -> p (u one)", p=128),
                    in_=elf[:])

                # ---- merge: exact top-20 of the 192 candidates ----
                cvf = sel.tile([128, NCAND], F32, tag="cvf")
                nc.vector.tensor_tensor(
                    out=cvf[:], in0=cv8[:], in1=ep192[:],
                    op=mybir.AluOpType.add)
                gvals = sel.tile([128, 24], F32, tag="gvals")
                gpos = sel.tile([128, 24], U32, tag="gpos")
                for r in range(3):
                    g8 = gvals[:, r * 8:(r + 1) * 8]
                    nc.vector.max(out=g8, in_=cvf[:])
                    nc.vector.max_index(
                        out=gpos[:, r * 8:(r + 1) * 8], in_max=g8,
                        in_values=cvf[:])
                    if r < 2:
                        nc.vector.match_replace(
                            out=cvf[:], in_to_replace=g8, in_values=cvf[:],
                            imm_value=NEG)
                # ---- exact fp32 rescore of the 24 candidates ----
                gposf = sel.tile([128, 24], F32, tag="gposf")
                nc.vector.tensor_copy(gposf[:], gpos[:])
                off2 = sel.tile([128, 24], F32, tag="off2")
                nc.vector.tensor_scalar(
                    off2[:], gposf[:], pr192[:], None, op0=mybir.AluOpType.add)
                offu2 = sel.tile([128, 24], U32, tag="offu2")
                nc.vector.tensor_copy(offu2[:], off2[:])
                el24 = sel.tile([128, 24], F32, tag="el24")
                nc.gpsimd.indirect_dma_start(
                    out=el24[:], out_offset=None, in_=elD[:],
                    in_offset=bass.IndirectOffsetOnAxis(ap=offu2[:], axis=0))
                el24u = sel.tile([128, 24], U32, tag="el24u")
                nc.vector.tensor_copy(el24u[:], el24[:])
                gmk = gbig.tile([128, 24 * (CK + 1)], F32, tag="gmk")
                nc.gpsimd.indirect_dma_start(
                    out=gmk[:].rearrange("p (k c) -> p k c", c=CK + 1),
                    out_offset=None, in_=mkT32[:],
                    in_offset=bass.IndirectOffsetOnAxis(ap=el24u[:], axis=0))
                # V rows for all 24 candidates, issued early so the gather
                # overlaps the rescore; non-top-20 slots get zero weight
                vTg = gbig.tile([128, 24 * 2 * CV], F16, tag="vTg")
                for hb in range(2):
                    nc.gpsimd.indirect_dma_start(
                        out=vTg[:, hb * 12 * 2 * CV:(hb + 1) * 12 * 2 * CV]
                        .rearrange("p (k c) -> p k c", c=2 * CV),
                        out_offset=None, in_=vTb[:],
                        in_offset=bass.IndirectOffsetOnAxis(
                            ap=el24u[:, hb * 12:(hb + 1) * 12], axis=0))
                nc.vector.tensor_tensor(
                    out=gmk[:].rearrange("p (k c) -> p k c", c=CK + 1),
                    in0=gmk[:].rearrange("p (k c) -> p k c", c=CK + 1),
                    in1=qrt[:].rearrange("p (u c) -> p u c", u=1)
                    .broadcast_to([128, 24, CK + 1]),
                    op=mybir.AluOpType.mult)
                av24 = sel.tile([128, 24], F32, tag="av24")
                nc.vector.tensor_reduce(
                    out=av24[:],
                    in_=gmk[:].rearrange("p (k c) -> p k c", c=CK + 1),
                    axis=mybir.AxisListType.X, op=mybir.AluOpType.add)
                nc.vector.tensor_tensor(
                    out=av24[:], in0=av24[:], in1=ep192[:, :24],
                    op=mybir.AluOpType.add)
                # ranked values (top-20 threshold), no positions needed
                av24c = sel.tile([128, 24], F32, tag="av24c")
                nc.vector.tensor_copy(av24c[:], av24[:])
                wvals = sel.tile([128, 24], F32, tag="wvals")
                for r in range(3):
                    w8 = wvals[:, r * 8:(r + 1) * 8]
                    nc.vector.max(out=w8, in_=av24c[:])
                    if r < 2:
                        nc.vector.match_replace(
                            out=av24c[:], in_to_replace=w8, in_values=av24c[:],
                            imm_value=NEG)

                # ---- masked softmax over all 24 slots (ranks >= 20 -> 0) ---
                mask = sel.tile([128, 24], F32, tag="mask")
                nc.vector.tensor_scalar(
                    mask[:], av24[:], wvals[:, 19:20], None,
                    op0=mybir.AluOpType.is_ge)
                negm40 = sel.tile([128, 1], F32, tag="negm40")
                nc.vector.tensor_scalar(
                    negm40[:], wvals[:, 0:1], -1.0, -40.0,
                    op0=mybir.AluOpType.mult, op1=mybir.AluOpType.add)
                # masked slots sit 40 below the kept ones -> exp ~ 0
                dms = sel.tile([128, 24], F32, tag="dms")
                nc.vector.scalar_tensor_tensor(
                    out=dms[:], in0=mask[:], scalar=40.0, in1=av24[:],
                    op0=mybir.AluOpType.mult, op1=mybir.AluOpType.add)
                wexp = sel.tile([128, 24], F32, tag="wexp")
                ssum = sel.tile([128, 1], F32, tag="ssum")
                nc.scalar.activation(
                    out=wexp[:], in_=dms[:],
                    func=mybir.ActivationFunctionType.Exp,
                    bias=negm40[:], scale=1.0, accum_out=ssum[:])
                rs = sel.tile([128, 1], F32, tag="rs")
                nc.vector.reciprocal(rs[:], ssum[:])
                wgt = sel.tile([128, 24], F32, tag="wgt")
                nc.vector.tensor_scalar(
                    wgt[:], wexp[:], rs[:], None, op0=mybir.AluOpType.mult)

                # ---- weighted readout over the 24 gathered V rows ----
                acc = gat.tile([128, 2 * CV], F32, tag="acc")
                nc.vector.tensor_scalar(
                    acc[:], vTg[:, 0:2 * CV], wgt[:, 0:1], None,
                    op0=mybir.AluOpType.mult)
                for k in range(1, 24):
                    nc.vector.scalar_tensor_tensor(
                        out=acc[:], in0=vTg[:, k * 2 * CV:(k + 1) * 2 * CV],
                        scalar=wgt[:, k:k + 1], in1=acc[:],
                        op0=mybir.AluOpType.mult, op1=mybir.AluOpType.add)
                nc.sync.dma_start(
                    out=out[t * 128:(t + 1) * 128, :], in_=acc[:])
    nc.finalize()
    _prog_cache["p"] = nc
    return nc


def _host_inputs(qk, mem_k, mem_v1, mem_v2, top_k=TOPK):
    qk = np.asarray(qk, dtype=np.float32)
    mem_k = np.asarray(mem_k, dtype=np.float32)
    mem_v1 = np.asarray(mem_v1, dtype=np.float32)
    mem_v2 = np.asarray(mem_v2, dtype=np.float32)

    q2 = qk.reshape(CK, Q)
    a = np.sum(mem_k[0] * mem_k[0], axis=0, dtype=np.float32)      # [NE]
    na = -0.125 * a
    nh = na.astype(np.float16).astype(np.float32)
    nl = (na - nh).astype(np.float16)
    mkB = np.concatenate(
        [mem_k[0].astype(np.float16), nh.astype(np.float16)[None, :],
         nl[None, :]], axis=0)                                      # [66, NE]
    vTb = np.concatenate(
        [mem_v1[0].T, mem_v2[0].T], axis=1).astype(np.float16)      # [NE, 512]
    mkT32 = np.ascontiguousarray(np.concatenate(
        [mem_k[0].T, na[:, None]], axis=1, dtype=np.float32))      # [NE, 65]
    prow512 = (np.arange(128, dtype=np.float32) * NCH).reshape(128, 1)
    prow192 = (np.arange(128, dtype=np.float32) * NCAND).reshape(128, 1)
    eps512 = np.broadcast_to(
        np.arange(NCH, dtype=np.float32) * EPS, (128, NCH)).copy()
    eps192 = np.broadcast_to(
        np.arange(NCAND, dtype=np.float32) * EPS, (128, NCAND)).copy()
    eps1536 = np.broadcast_to(
        1.0 + (np.arange(NSEL * CW, dtype=np.float32) % (2 * CW)) * 2.0 ** -21,
        (128, NSEL * CW)).astype(np.float32).copy()

    in_maps = []
    for c in range(NC):
        sl = slice(c * Q_LOC, (c + 1) * Q_LOC)
        qs = 0.25 * q2[:, sl]
        qTb = np.concatenate(
            [qs.astype(np.float16), np.ones((2, Q_LOC), np.float16)],
            axis=0)                                                 # [66, 512]
        qrI = np.ascontiguousarray(np.concatenate(
            [qs.T, np.ones((Q_LOC, 1), np.float32)],
            axis=1, dtype=np.float32))                              # [512, 65]
        in_maps.append({
            "qTb": qTb, "mkB": mkB, "vTb": vTb,
            "mkT32": mkT32, "qrI": qrI,
            "prow512": prow512, "prow192": prow192,
            "eps512": eps512, "eps192": eps192, "eps1536": eps1536,
        })
    return in_maps


def _assemble_output(outs):
    full = np.concatenate(outs, axis=0)
    return np.ascontiguousarray(full.T).reshape(1, 2 * CV, H, W)


def kernel(qk, mem_k, mem_v1, mem_v2, top_k):
    assert int(top_k) == TOPK
    in_maps = _host_inputs(qk, mem_k, mem_v1, mem_v2)
    nc = _build_program()
    res = None
    for attempt in range(3):
        try:
            res = run_bass_kernel_spmd(nc, in_maps, core_ids=list(range(NC)))
            break
        except Exception:
            # transient device-unrecoverable states clear on the next attempt
            if attempt == 2:
                raise
            time.sleep(2.0)
    return _assemble_output([res.results[c]["out"] for c in range(NC)])


# revision 57
# speedup vs baseline: 1.1795x; 1.0079x over previous
import sys, time
sys.path.insert(0, "/opt/trn_rl_repo")
import numpy as np
from concourse import bass, bacc, mybir, tile
from concourse.bass_utils import run_bass_kernel_spmd

# Problem constants (nn_Memory_88656714925588)
B, CK, CV = 1, 64, 256
H, W, T = 64, 64, 8
NE = H * W * T            # 32768 memory elements
Q = H * W * 64 // 64      # 4096 queries
NC = 8                    # cores
Q_LOC = Q // NC           # 512 queries per core (query-sharded)
NQT = Q_LOC // 128        # 4 query tiles per core
TOPK = 20
CW = 64                   # chunk width for the screen
NCH = NE // CW            # 512 chunks per query row
NSEL = 24                 # chunks selected per query (>= 20 guarantees coverage)
NPR = NSEL // 2           # rescan processes chunk pairs
NCAND = NPR * 8           # 96 candidates after per-pair top-8
NSLICE = 8                # 4096-column slices per tile
SLW = NE // NSLICE        # 4096
F32 = mybir.dt.float32
F16 = mybir.dt.float16
U32 = mybir.dt.uint32
NEG = -1e30
EPS = 2.0 ** -21

_prog_cache = {}


def _build_program():
    if "p" in _prog_cache:
        return _prog_cache["p"]
    nc = bacc.Bacc()
    qTb = nc.dram_tensor("qTb", [CK + 2, Q_LOC], F16, kind="ExternalInput")
    mkB = nc.dram_tensor("mkB", [CK + 2, NE], F16, kind="ExternalInput")
    eps1536 = nc.dram_tensor(
        "eps1536", [128, NSEL * CW], F32, kind="ExternalInput")
    vTb = nc.dram_tensor("vTb", [NE, 2 * CV], F16, kind="ExternalInput")
    mkT32 = nc.dram_tensor("mkT32", [NE, CK + 1], F32, kind="ExternalInput")
    qrI = nc.dram_tensor("qrI", [Q_LOC, CK + 1], F32, kind="ExternalInput")
    prow512 = nc.dram_tensor("prow512", [128, 1], F32, kind="ExternalInput")
    prow192 = nc.dram_tensor("prow192", [128, 1], F32, kind="ExternalInput")
    eps512 = nc.dram_tensor("eps512", [128, NCH], F32, kind="ExternalInput")
    eps192 = nc.dram_tensor("eps192", [128, NCAND], F32, kind="ExternalInput")
    out = nc.dram_tensor("out", [Q_LOC, 2 * CV], F32, kind="ExternalOutput")

    with tile.TileContext(nc) as tc:
        with tc.tile_pool(name="cst", bufs=1) as cst, \
             tc.tile_pool(name="aff", bufs=3) as affp, \
             tc.tile_pool(name="tree", bufs=2) as tre, \
             tc.tile_pool(name="sel", bufs=2) as sel, \
             tc.tile_pool(name="gat", bufs=2) as gat, \
             tc.tile_pool(name="gbig", bufs=1) as gbig, \
             tc.tile_pool(name="psum", bufs=2, space="PSUM") as psum, \
             tc.tile_pool(name="dram", bufs=2, space="DRAM") as dram:

            qt = cst.tile([CK + 2, Q_LOC], F16)
            mkt = cst.tile([CK + 2, NE], F16)
            # small inputs first so the first matmul isn't queued behind
            # the big mk transfers
            nc.sync.dma_start(out=qt[:], in_=qTb[:])
            ep1536 = cst.tile([128, NSEL * CW], F32)
            nc.gpsimd.dma_start(out=ep1536[:], in_=eps1536[:])
            pr512 = cst.tile([128, 1], F32)
            nc.sync.dma_start(out=pr512[:], in_=prow512[:])
            pr192 = cst.tile([128, 1], F32)
            nc.sync.dma_start(out=pr192[:], in_=prow192[:])
            ep512 = cst.tile([128, NCH], F32)
            nc.gpsimd.dma_start(out=ep512[:], in_=eps512[:])
            ep192 = cst.tile([128, NCAND], F32)
            nc.gpsimd.dma_start(out=ep192[:], in_=eps192[:])
            # chunked mk load, split across queues; tiny first chunk so the
            # first matmul can start almost immediately
            bounds = [0, 512, 1024, 2048, 4096] + \
                [SLW * i for i in range(2, NSLICE + 1)]
            for ci in range(len(bounds) - 1):
                eng = nc.sync if ci % 2 == 0 else nc.gpsimd
                eng.dma_start(
                    out=mkt[:, bounds[ci]:bounds[ci + 1]],
                    in_=mkB[:, bounds[ci]:bounds[ci + 1]])

            for t in range(NQT):
                qrt = sel.tile([128, CK + 1], F32, tag="qrt")
                nc.sync.dma_start(
                    out=qrt[:], in_=qrI[t * 128:(t + 1) * 128, :])
                affsD = dram.tile([128 * NCH, CW], F16, tag="affsD")
                affsDv = affsD[:].rearrange("(p c) w -> p (c w)", p=128)
                elD = dram.tile([128 * NCAND, 1], F32, tag="elD")
                cmax = tre.tile([128, NCH], F16, tag="cmax")

                for s in range(NSLICE):
                    aff4 = affp.tile([128, SLW], F16, tag="aff4")
                    for h in range(2):
                        ph = psum.tile([128, 2048], F32, tag="ph")
                        for c in range(4):
                            col = s * SLW + h * 2048 + c * 512
                            nc.tensor.matmul(
                                out=ph[:, c * 512:(c + 1) * 512],
                                lhsT=qt[:, t * 128:(t + 1) * 128],
                                rhs=mkt[:, col:col + 512],
                                start=True, stop=True)
                        nc.scalar.activation(
                            out=aff4[:, h * 2048:(h + 1) * 2048], in_=ph[:],
                            func=mybir.ActivationFunctionType.Copy)
                    # stage this slice to DRAM for the per-query rescan gathers
                    eng = nc.gpsimd if s in (2, 5) else nc.sync
                    eng.dma_start(
                        out=affsDv[:, s * SLW:(s + 1) * SLW], in_=aff4[:])
                    # chunk-local pairwise-max tree: 4096 -> 64 chunk maxima
                    a3 = aff4[:].rearrange("p (g w) -> p g w", w=CW)
                    t1 = tre.tile([128, 2048], F16, tag="t1")
                    nc.vector.tensor_tensor(
                        out=t1[:].rearrange("p (g w) -> p g w", w=32),
                        in0=a3[:, :, 0:32], in1=a3[:, :, 32:64],
                        op=mybir.AluOpType.max)
                    t2 = tre.tile([128, 1024], F16, tag="t2")
                    nc.vector.tensor_tensor(
                        out=t2[:].rearrange("p (g w) -> p g w", w=16),
                        in0=t1[:].rearrange("p (g w) -> p g w", w=32)[:, :, 0:16],
                        in1=t1[:].rearrange("p (g w) -> p g w", w=32)[:, :, 16:32],
                        op=mybir.AluOpType.max)
                    t3 = tre.tile([128, 512], F16, tag="t3")
                    nc.vector.tensor_tensor(
                        out=t3[:].rearrange("p (g w) -> p g w", w=8),
                        in0=t2[:].rearrange("p (g w) -> p g w", w=16)[:, :, 0:8],
                        in1=t2[:].rearrange("p (g w) -> p g w", w=16)[:, :, 8:16],
                        op=mybir.AluOpType.max)
                    t4 = tre.tile([128, 256], F16, tag="t4")
                    nc.vector.tensor_tensor(
                        out=t4[:].rearrange("p (g w) -> p g w", w=4),
                        in0=t3[:].rearrange("p (g w) -> p g w", w=8)[:, :, 0:4],
                        in1=t3[:].rearrange("p (g w) -> p g w", w=8)[:, :, 4:8],
                        op=mybir.AluOpType.max)
                    t5 = tre.tile([128, 128], F16, tag="t5")
                    nc.vector.tensor_tensor(
                        out=t5[:].rearrange("p (g w) -> p g w", w=2),
                        in0=t4[:].rearrange("p (g w) -> p g w", w=4)[:, :, 0:2],
                        in1=t4[:].rearrange("p (g w) -> p g w", w=4)[:, :, 2:4],
                        op=mybir.AluOpType.max)
                    nc.vector.tensor_tensor(
                        out=cmax[:, s * 64:(s + 1) * 64],
                        in0=t5[:].rearrange("p (g w) -> p g w", w=2)[:, :, 0],
                        in1=t5[:].rearrange("p (g w) -> p g w", w=2)[:, :, 1],
                        op=mybir.AluOpType.max)

                # ---- select top-NSEL chunks per query (tie-free in f32) ----
                cmaxf = sel.tile([128, NCH], F32, tag="cmaxf")
                nc.vector.scalar_tensor_tensor(
                    out=cmaxf[:], in0=cmax[:], scalar=1.0, in1=ep512[:],
                    op0=mybir.AluOpType.mult, op1=mybir.AluOpType.add)
                cidu = sel.tile([128, NSEL], U32, tag="cidu")
                m8 = sel.tile([128, 8], F32, tag="m8")
                for r in range(NSEL // 8):
                    nc.vector.max(out=m8[:], in_=cmaxf[:])
                    nc.vector.max_index(
                        out=cidu[:, r * 8:(r + 1) * 8], in_max=m8[:],
                        in_values=cmaxf[:])
                    if r < NSEL // 8 - 1:
                        nc.vector.match_replace(
                            out=cmaxf[:], in_to_replace=m8[:],
                            in_values=cmaxf[:], imm_value=NEG)
                cidf = sel.tile([128, NSEL], F32, tag="cidf")
                nc.vector.tensor_copy(cidf[:], cidu[:])
                offf = sel.tile([128, NSEL], F32, tag="offf")
                nc.vector.tensor_scalar(
                    offf[:], cidf[:], pr512[:], None, op0=mybir.AluOpType.add)
                offu = sel.tile([128, NSEL], U32, tag="offu")
                nc.vector.tensor_copy(offu[:], offf[:])

                # ---- gather the selected chunks, rescan for top-8 each ----
                g24 = gbig.tile([128, NSEL * CW], F16, tag="g24")
                nc.gpsimd.indirect_dma_start(
                    out=g24[:].rearrange("p (k w) -> p k w", w=CW),
                    out_offset=None, in_=affsD[:],
                    in_offset=bass.IndirectOffsetOnAxis(ap=offu[:], axis=0))
                # multiplicative positional eps: v' = v*(1 + pos*2^-19).
                # v is on the f16 grid, so round-to-f16 recovers v and the
                # ratio recovers pos -- no max_index pass needed.
                g24f = gbig.tile([128, NSEL * CW], F32, tag="g24f")
                nc.vector.tensor_tensor(
                    out=g24f[:], in0=g24[:], in1=ep1536[:],
                    op=mybir.AluOpType.mult)
                cv8 = sel.tile([128, NCAND], F32, tag="cv8")
                for j in range(NPR):
                    nc.vector.max(
                        out=cv8[:, j * 8:(j + 1) * 8],
                        in_=g24f[:, j * 2 * CW:(j + 1) * 2 * CW])
                vq16 = sel.tile([128, NCAND], F16, tag="vq16")
                nc.vector.tensor_copy(vq16[:], cv8[:])
                # denominator guard: +-1e-30 by sign so v=0 can't divide by 0
                sg = sel.tile([128, NCAND], F32, tag="sg")
                nc.vector.tensor_scalar(
                    sg[:], vq16[:], 0.0, None, op0=mybir.AluOpType.is_ge)
                nc.vector.tensor_scalar(
                    sg[:], sg[:], 2e-30, -1e-30,
                    op0=mybir.AluOpType.mult, op1=mybir.AluOpType.add)
                vq2 = sel.tile([128, NCAND], F32, tag="vq2")
                nc.vector.tensor_tensor(
                    out=vq2[:], in0=vq16[:], in1=sg[:],
                    op=mybir.AluOpType.add)
                pf = sel.tile([128, NCAND], F32, tag="pf")
                nc.vector.tensor_tensor(
                    out=pf[:], in0=cv8[:], in1=vq2[:],
                    op=mybir.AluOpType.divide)
                nc.vector.tensor_scalar(
                    pf[:], pf[:], float(2 ** 21), -float(2 ** 21) + 0.5,
                    op0=mybir.AluOpType.mult, op1=mybir.AluOpType.add)
                nc.vector.tensor_scalar(
                    pf[:], pf[:], 0.0, None, op0=mybir.AluOpType.max)
                nc.vector.tensor_scalar(
                    pf[:], pf[:], 127.9, None, op0=mybir.AluOpType.min)
                # pair decode: el = cid[2j]*64 + pf, or the odd chunk's base
                # when pf falls in the second half of the gathered pair
                cid3 = cidf[:].rearrange("p (j two) -> p j two", two=2)
                cdm = sel.tile([128, NPR], F32, tag="cdm")
                nc.vector.tensor_tensor(
                    out=cdm[:], in0=cid3[:, :, 1], in1=cid3[:, :, 0],
                    op=mybir.AluOpType.subtract)
                nc.vector.tensor_scalar(
                    cdm[:], cdm[:], float(CW), -float(CW),
                    op0=mybir.AluOpType.mult, op1=mybir.AluOpType.add)
                ge = sel.tile([128, NCAND], F32, tag="ge")
                nc.vector.tensor_scalar(
                    ge[:], pf[:], float(CW), None, op0=mybir.AluOpType.is_ge)
                nc.vector.tensor_tensor(
                    out=ge[:].rearrange("p (j r) -> p j r", r=8),
                    in0=ge[:].rearrange("p (j r) -> p j r", r=8),
                    in1=cdm[:].rearrange("p (j u) -> p j u", u=1)
                    .broadcast_to([128, NPR, 8]),
                    op=mybir.AluOpType.mult)
                elf = sel.tile([128, NCAND], F32, tag="elf")
                nc.vector.scalar_tensor_tensor(
                    out=elf[:].rearrange("p (j r) -> p j r", r=8),
                    in0=cid3[:, :, 0:1].broadcast_to([128, NPR, 8]),
                    scalar=float(CW),
                    in1=pf[:].rearrange("p (j r) -> p j r", r=8),
                    op0=mybir.AluOpType.mult, op1=mybir.AluOpType.add)
                nc.vector.tensor_tensor(
                    out=elf[:], in0=elf[:], in1=ge[:],
                    op=mybir.AluOpType.add)
                nc.sync.dma_start(
                    out=elD[:].rearrange("(p u) one -> p (u one)", p=128),
                    in_=elf[:])

                # ---- merge: exact top-20 of the 192 candidates ----
                cvf = sel.tile([128, NCAND], F32, tag="cvf")
                nc.vector.tensor_tensor(
                    out=cvf[:], in0=cv8[:], in1=ep192[:],
                    op=mybir.AluOpType.add)
                gvals = sel.tile([128, 24], F32, tag="gvals")
                gpos = sel.tile([128, 24], U32, tag="gpos")
                for r in range(3):
                    g8 = gvals[:, r * 8:(r + 1) * 8]
                    nc.vector.max(out=g8, in_=cvf[:])
                    nc.vector.max_index(
                        out=gpos[:, r * 8:(r + 1) * 8], in_max=g8,
                        in_values=cvf[:])
                    if r < 2:
                        nc.vector.match_replace(
                            out=cvf[:], in_to_replace=g8, in_values=cvf[:],
                            imm_value=NEG)
                # ---- exact fp32 rescore of the 24 candidates ----
                gposf = sel.tile([128, 24], F32, tag="gposf")
                nc.vector.tensor_copy(gposf[:], gpos[:])
                off2 = sel.tile([128, 24], F32, tag="off2")
                nc.vector.tensor_scalar(
                    off2[:], gposf[:], pr192[:], None, op0=mybir.AluOpType.add)
                offu2 = sel.tile([128, 24], U32, tag="offu2")
                nc.vector.tensor_copy(offu2[:], off2[:])
                el24 = sel.tile([128, 24], F32, tag="el24")
                nc.gpsimd.indirect_dma_start(
                    out=el24[:], out_offset=None, in_=elD[:],
                    in_offset=bass.IndirectOffsetOnAxis(ap=offu2[:], axis=0))
                el24u = sel.tile([128, 24], U32, tag="el24u")
                nc.vector.tensor_copy(el24u[:], el24[:])
                gmk = gbig.tile([128, 24 * (CK + 1)], F32, tag="gmk")
                nc.gpsimd.indirect_dma_start(
                    out=gmk[:].rearrange("p (k c) -> p k c", c=CK + 1),
                    out_offset=None, in_=mkT32[:],
                    in_offset=bass.IndirectOffsetOnAxis(ap=el24u[:], axis=0))
                # V rows for all 24 candidates, issued early so the gather
                # overlaps the rescore; non-top-20 slots get zero weight
                vTg = gbig.tile([128, 24 * 2 * CV], F16, tag="vTg")
                for hb in range(2):
                    nc.gpsimd.indirect_dma_start(
                        out=vTg[:, hb * 12 * 2 * CV:(hb + 1) * 12 * 2 * CV]
                        .rearrange("p (k c) -> p k c", c=2 * CV),
                        out_offset=None, in_=vTb[:],
                        in_offset=bass.IndirectOffsetOnAxis(
                            ap=el24u[:, hb * 12:(hb + 1) * 12], axis=0))
                nc.vector.tensor_tensor(
                    out=gmk[:].rearrange("p (k c) -> p k c", c=CK + 1),
                    in0=gmk[:].rearrange("p (k c) -> p k c", c=CK + 1),
                    in1=qrt[:].rearrange("p (u c) -> p u c", u=1)
                    .broadcast_to([128, 24, CK + 1]),
                    op=mybir.AluOpType.mult)
                av24 = sel.tile([128, 24], F32, tag="av24")
                nc.vector.tensor_reduce(
                    out=av24[:],
                    in_=gmk[:].rearrange("p (k c) -> p k c", c=CK + 1),
                    axis=mybir.AxisListType.X, op=mybir.AluOpType.add)
                nc.vector.tensor_tensor(
                    out=av24[:], in0=av24[:], in1=ep192[:, :24],
                    op=mybir.AluOpType.add)
                # ranked values (top-20 threshold), no positions needed
                av24c = sel.tile([128, 24], F32, tag="av24c")
                nc.vector.tensor_copy(av24c[:], av24[:])
                wvals = sel.tile([128, 24], F32, tag="wvals")
                for r in range(3):
                    w8 = wvals[:, r * 8:(r + 1) * 8]
                    nc.vector.max(out=w8, in_=av24c[:])
                    if r < 2:
                        nc.vector.match_replace(
                            out=av24c[:], in_to_replace=w8, in_values=av24c[:],
                            imm_value=NEG)

                # ---- masked softmax over all 24 slots (ranks >= 20 -> 0) ---
                mask = sel.tile([128, 24], F32, tag="mask")
                nc.vector.tensor_scalar(
                    mask[:], av24[:], wvals[:, 19:20], None,
                    op0=mybir.AluOpType.is_ge)
                negm40 = sel.tile([128, 1], F32, tag="negm40")
                nc.vector.tensor_scalar(
                    negm40[:], wvals[:, 0:1], -1.0, -40.0,
                    op0=mybir.AluOpType.mult, op1=mybir.AluOpType.add)
                # masked slots sit 40 below the kept ones -> exp ~ 0
                dms = sel.tile([128, 24], F32, tag="dms")
                nc.vector.scalar_tensor_tensor(
                    out=dms[:], in0=mask[:], scalar=40.0, in1=av24[:],
                    op0=mybir.AluOpType.mult, op1=mybir.AluOpType.add)
                wexp = sel.tile([128, 24], F32, tag="wexp")
                ssum = sel.tile([128, 1], F32, tag="ssum")
                nc.scalar.activation(
                    out=wexp[:], in_=dms[:],
                    func=mybir.ActivationFunctionType.Exp,
                    bias=negm40[:], scale=1.0, accum_out=ssum[:])
                rs = sel.tile([128, 1], F32, tag="rs")
                nc.vector.reciprocal(rs[:], ssum[:])
                wgt = sel.tile([128, 24], F32, tag="wgt")
                nc.vector.tensor_scalar(
                    wgt[:], wexp[:], rs[:], None, op0=mybir.AluOpType.mult)

                # ---- weighted readout over the 24 gathered V rows ----
                # slots 12..23 are scaled on the Activation engine and
                # pair-summed; DVE accumulates slots 0..11 directly
                terms = gbig.tile([128, 12 * 2 * CV], F16, tag="terms")
                for k in range(12, 24):
                    j = k - 12
                    nc.scalar.activation(
                        out=terms[:, j * 2 * CV:(j + 1) * 2 * CV],
                        in_=vTg[:, k * 2 * CV:(k + 1) * 2 * CV],
                        func=mybir.ActivationFunctionType.Copy,
                        scale=wgt[:, k:k + 1])
                acc = gat.tile([128, 2 * CV], F32, tag="acc")
                nc.vector.tensor_scalar(
                    acc[:], vTg[:, 0:2 * CV], wgt[:, 0:1], None,
                    op0=mybir.AluOpType.mult)
                for k in range(1, 12):
                    nc.vector.scalar_tensor_tensor(
                        out=acc[:], in0=vTg[:, k * 2 * CV:(k + 1) * 2 * CV],
                        scalar=wgt[:, k:k + 1], in1=acc[:],
                        op0=mybir.AluOpType.mult, op1=mybir.AluOpType.add)
                s6 = gat.tile([128, 6 * 2 * CV], F16, tag="s6")
                nc.vector.tensor_tensor(
                    out=s6[:], in0=terms[:, :6 * 2 * CV],
                    in1=terms[:, 6 * 2 * CV:], op=mybir.AluOpType.add)
                s3 = gat.tile([128, 3 * 2 * CV], F16, tag="s3")
                nc.vector.tensor_tensor(
                    out=s3[:], in0=s6[:, :3 * 2 * CV],
                    in1=s6[:, 3 * 2 * CV:], op=mybir.AluOpType.add)
                nc.vector.tensor_tensor(
                    out=acc[:], in0=s3[:, 0:2 * CV], in1=acc[:],
                    op=mybir.AluOpType.add)
                nc.vector.tensor_tensor(
                    out=s3[:, 0:2 * CV], in0=s3[:, 2 * CV:2 * 2 * CV],
                    in1=s3[:, 2 * 2 * CV:], op=mybir.AluOpType.add)
                nc.vector.tensor_tensor(
                    out=acc[:], in0=s3[:, 0:2 * CV], in1=acc[:],
                    op=mybir.AluOpType.add)
                nc.sync.dma_start(
                    out=out[t * 128:(t + 1) * 128, :], in_=acc[:])
    nc.finalize()
    _prog_cache["p"] = nc
    return nc


def _host_inputs(qk, mem_k, mem_v1, mem_v2, top_k=TOPK):
    qk = np.asarray(qk, dtype=np.float32)
    mem_k = np.asarray(mem_k, dtype=np.float32)
    mem_v1 = np.asarray(mem_v1, dtype=np.float32)
    mem_v2 = np.asarray(mem_v2, dtype=np.float32)

    q2 = qk.reshape(CK, Q)
    a = np.sum(mem_k[0] * mem_k[0], axis=0, dtype=np.float32)      # [NE]
    na = -0.125 * a
    nh = na.astype(np.float16).astype(np.float32)
    nl = (na - nh).astype(np.float16)
    mkB = np.concatenate(
        [mem_k[0].astype(np.float16), nh.astype(np.float16)[None, :],
         nl[None, :]], axis=0)                                      # [66, NE]
    vTb = np.concatenate(
        [mem_v1[0].T, mem_v2[0].T], axis=1).astype(np.float16)      # [NE, 512]
    mkT32 = np.ascontiguousarray(np.concatenate(
        [mem_k[0].T, na[:, None]], axis=1, dtype=np.float32))      # [NE, 65]
    prow512 = (np.arange(128, dtype=np.float32) * NCH).reshape(128, 1)
    prow192 = (np.arange(128, dtype=np.float32) * NCAND).reshape(128, 1)
    eps512 = np.broadcast_to(
        np.arange(NCH, dtype=np.float32) * EPS, (128, NCH)).copy()
    eps192 = np.broadcast_to(
        np.arange(NCAND, dtype=np.float32) * EPS, (128, NCAND)).copy()
    eps1536 = np.broadcast_to(
        1.0 + (np.arange(NSEL * CW, dtype=np.float32) % (2 * CW)) * 2.0 ** -21,
        (128, NSEL * CW)).astype(np.float32).copy()

    in_maps = []
    for c in range(NC):
        sl = slice(c * Q_LOC, (c + 1) * Q_LOC)
        qs = 0.25 * q2[:, sl]
        qTb = np.concatenate(
            [qs.astype(np.float16), np.ones((2, Q_LOC), np.float16)],
            axis=0)                                                 # [66, 512]
        qrI = np.ascontiguousarray(np.concatenate(
            [qs.T, np.ones((Q_LOC, 1), np.float32)],
            axis=1, dtype=np.float32))                              # [512, 65]
        in_maps.append({
            "qTb": qTb, "mkB": mkB, "vTb": vTb,
            "mkT32": mkT32, "qrI": qrI,
            "prow512": prow512, "prow192": prow192,
            "eps512": eps512, "eps192": eps192, "eps1536": eps1536,
        })
    return in_maps


def _assemble_output(outs):
    full = np.concatenate(outs, axis=0)
    return np.ascontiguousarray(full.T).reshape(1, 2 * CV, H, W)


def kernel(qk, mem_k, mem_v1, mem_v2, top_k):
    assert int(top_k) == TOPK
    in_maps = _host_inputs(qk, mem_k, mem_v1, mem_v2)
    nc = _build_program()
    res = None
    for attempt in range(3):
        try:
            res = run_bass_kernel_spmd(nc, in_maps, core_ids=list(range(NC)))
            break
        except Exception:
            # transient device-unrecoverable states clear on the next attempt
            if attempt == 2:
                raise
            time.sleep(2.0)
    return _assemble_output([res.results[c]["out"] for c in range(NC)])


# revision 59
# speedup vs baseline: 1.1944x; 1.0127x over previous
import sys, time
sys.path.insert(0, "/opt/trn_rl_repo")
import numpy as np
from concourse import bass, bacc, mybir, tile
from concourse.bass_utils import run_bass_kernel_spmd

# Problem constants (nn_Memory_88656714925588)
B, CK, CV = 1, 64, 256
H, W, T = 64, 64, 8
NE = H * W * T            # 32768 memory elements
Q = H * W * 64 // 64      # 4096 queries
NC = 8                    # cores
Q_LOC = Q // NC           # 512 queries per core (query-sharded)
NQT = Q_LOC // 128        # 4 query tiles per core
TOPK = 20
CW = 64                   # chunk width for the screen
NCH = NE // CW            # 512 chunks per query row
NSEL = 24                 # chunks selected per query (>= 20 guarantees coverage)
NPR = NSEL // 2           # rescan processes chunk pairs
NCAND = NPR * 8           # 96 candidates after per-pair top-8
NSLICE = 8                # 4096-column slices per tile
SLW = NE // NSLICE        # 4096
F32 = mybir.dt.float32
F16 = mybir.dt.float16
U32 = mybir.dt.uint32
NEG = -1e30
EPS = 2.0 ** -21

_prog_cache = {}


def _build_program():
    if "p" in _prog_cache:
        return _prog_cache["p"]
    nc = bacc.Bacc()
    qTb = nc.dram_tensor("qTb", [CK + 2, Q_LOC], F16, kind="ExternalInput")
    mkB = nc.dram_tensor("mkB", [CK + 2, NE], F16, kind="ExternalInput")
    eps1536 = nc.dram_tensor(
        "eps1536", [128, NSEL * CW], F32, kind="ExternalInput")
    vTb = nc.dram_tensor("vTb", [NE, 2 * CV], F16, kind="ExternalInput")
    mkT32 = nc.dram_tensor("mkT32", [NE, CK + 1], F32, kind="ExternalInput")
    qrI = nc.dram_tensor("qrI", [Q_LOC, CK + 1], F32, kind="ExternalInput")
    prow512 = nc.dram_tensor("prow512", [128, 1], F32, kind="ExternalInput")
    prow192 = nc.dram_tensor("prow192", [128, 1], F32, kind="ExternalInput")
    eps512 = nc.dram_tensor("eps512", [128, NCH], F32, kind="ExternalInput")
    eps192 = nc.dram_tensor("eps192", [128, NCAND], F32, kind="ExternalInput")
    out = nc.dram_tensor("out", [Q_LOC, 2 * CV], F32, kind="ExternalOutput")

    with tile.TileContext(nc) as tc:
        with tc.tile_pool(name="cst", bufs=1) as cst, \
             tc.tile_pool(name="aff", bufs=3) as affp, \
             tc.tile_pool(name="tree", bufs=2) as tre, \
             tc.tile_pool(name="sel", bufs=2) as sel, \
             tc.tile_pool(name="gat", bufs=2) as gat, \
             tc.tile_pool(name="gbig", bufs=1) as gbig, \
             tc.tile_pool(name="psum", bufs=2, space="PSUM") as psum, \
             tc.tile_pool(name="dram", bufs=2, space="DRAM") as dram:

            qt = cst.tile([CK + 2, Q_LOC], F16)
            mkt = cst.tile([CK + 2, NE], F16)
            # small inputs first so the first matmul isn't queued behind
            # the big mk transfers
            nc.sync.dma_start(out=qt[:], in_=qTb[:])
            ep1536 = cst.tile([128, NSEL * CW], F32)
            nc.gpsimd.dma_start(out=ep1536[:], in_=eps1536[:])
            pr512 = cst.tile([128, 1], F32)
            nc.sync.dma_start(out=pr512[:], in_=prow512[:])
            pr192 = cst.tile([128, 1], F32)
            nc.sync.dma_start(out=pr192[:], in_=prow192[:])
            ep512 = cst.tile([128, NCH], F32)
            nc.gpsimd.dma_start(out=ep512[:], in_=eps512[:])
            ep192 = cst.tile([128, NCAND], F32)
            nc.gpsimd.dma_start(out=ep192[:], in_=eps192[:])
            # chunked mk load, split across queues; tiny first chunk so the
            # first matmul can start almost immediately
            bounds = [0, 512, 1024, 2048, 4096] + \
                [SLW * i for i in range(2, NSLICE + 1)]
            for ci in range(len(bounds) - 1):
                eng = nc.sync if ci % 2 == 0 else nc.gpsimd
                eng.dma_start(
                    out=mkt[:, bounds[ci]:bounds[ci + 1]],
                    in_=mkB[:, bounds[ci]:bounds[ci + 1]])

            for t in range(NQT):
                qrt = sel.tile([128, CK + 1], F32, tag="qrt")
                nc.sync.dma_start(
                    out=qrt[:], in_=qrI[t * 128:(t + 1) * 128, :])
                affsD = dram.tile([128 * NCH, CW], F16, tag="affsD")
                affsDv = affsD[:].rearrange("(p c) w -> p (c w)", p=128)
                elD = dram.tile([128 * NCAND, 1], F32, tag="elD")
                cmax = tre.tile([128, NCH], F16, tag="cmax")

                for s in range(NSLICE):
                    aff4 = affp.tile([128, SLW], F16, tag="aff4")
                    for h in range(2):
                        ph = psum.tile([128, 2048], F32, tag="ph")
                        for c in range(4):
                            col = s * SLW + h * 2048 + c * 512
                            nc.tensor.matmul(
                                out=ph[:, c * 512:(c + 1) * 512],
                                lhsT=qt[:, t * 128:(t + 1) * 128],
                                rhs=mkt[:, col:col + 512],
                                start=True, stop=True)
                        nc.scalar.activation(
                            out=aff4[:, h * 2048:(h + 1) * 2048], in_=ph[:],
                            func=mybir.ActivationFunctionType.Copy)
                    # stage this slice to DRAM for the per-query rescan gathers
                    eng = nc.gpsimd if (s in (2, 5) and t < NQT - 1) \
                        else nc.sync
                    eng.dma_start(
                        out=affsDv[:, s * SLW:(s + 1) * SLW], in_=aff4[:])
                    # chunk-local pairwise-max tree: 4096 -> 64 chunk maxima
                    a3 = aff4[:].rearrange("p (g w) -> p g w", w=CW)
                    t1 = tre.tile([128, 2048], F16, tag="t1")
                    nc.vector.tensor_tensor(
                        out=t1[:].rearrange("p (g w) -> p g w", w=32),
                        in0=a3[:, :, 0:32], in1=a3[:, :, 32:64],
                        op=mybir.AluOpType.max)
                    t2 = tre.tile([128, 1024], F16, tag="t2")
                    nc.vector.tensor_tensor(
                        out=t2[:].rearrange("p (g w) -> p g w", w=16),
                        in0=t1[:].rearrange("p (g w) -> p g w", w=32)[:, :, 0:16],
                        in1=t1[:].rearrange("p (g w) -> p g w", w=32)[:, :, 16:32],
                        op=mybir.AluOpType.max)
                    t3 = tre.tile([128, 512], F16, tag="t3")
                    nc.vector.tensor_tensor(
                        out=t3[:].rearrange("p (g w) -> p g w", w=8),
                        in0=t2[:].rearrange("p (g w) -> p g w", w=16)[:, :, 0:8],
                        in1=t2[:].rearrange("p (g w) -> p g w", w=16)[:, :, 8:16],
                        op=mybir.AluOpType.max)
                    t4 = tre.tile([128, 256], F16, tag="t4")
                    nc.vector.tensor_tensor(
                        out=t4[:].rearrange("p (g w) -> p g w", w=4),
                        in0=t3[:].rearrange("p (g w) -> p g w", w=8)[:, :, 0:4],
                        in1=t3[:].rearrange("p (g w) -> p g w", w=8)[:, :, 4:8],
                        op=mybir.AluOpType.max)
                    t5 = tre.tile([128, 128], F16, tag="t5")
                    nc.vector.tensor_tensor(
                        out=t5[:].rearrange("p (g w) -> p g w", w=2),
                        in0=t4[:].rearrange("p (g w) -> p g w", w=4)[:, :, 0:2],
                        in1=t4[:].rearrange("p (g w) -> p g w", w=4)[:, :, 2:4],
                        op=mybir.AluOpType.max)
                    nc.vector.tensor_tensor(
                        out=cmax[:, s * 64:(s + 1) * 64],
                        in0=t5[:].rearrange("p (g w) -> p g w", w=2)[:, :, 0],
                        in1=t5[:].rearrange("p (g w) -> p g w", w=2)[:, :, 1],
                        op=mybir.AluOpType.max)

                # ---- select top-NSEL chunks per query (tie-free in f32) ----
                cmaxf = sel.tile([128, NCH], F32, tag="cmaxf")
                nc.vector.scalar_tensor_tensor(
                    out=cmaxf[:], in0=cmax[:], scalar=1.0, in1=ep512[:],
                    op0=mybir.AluOpType.mult, op1=mybir.AluOpType.add)
                cidu = sel.tile([128, NSEL], U32, tag="cidu")
                m8 = sel.tile([128, 8], F32, tag="m8")
                for r in range(NSEL // 8):
                    nc.vector.max(out=m8[:], in_=cmaxf[:])
                    nc.vector.max_index(
                        out=cidu[:, r * 8:(r + 1) * 8], in_max=m8[:],
                        in_values=cmaxf[:])
                    if r < NSEL // 8 - 1:
                        nc.vector.match_replace(
                            out=cmaxf[:], in_to_replace=m8[:],
                            in_values=cmaxf[:], imm_value=NEG)
                cidf = sel.tile([128, NSEL], F32, tag="cidf")
                nc.vector.tensor_copy(cidf[:], cidu[:])
                offf = sel.tile([128, NSEL], F32, tag="offf")
                nc.vector.tensor_scalar(
                    offf[:], cidf[:], pr512[:], None, op0=mybir.AluOpType.add)
                offu = sel.tile([128, NSEL], U32, tag="offu")
                nc.vector.tensor_copy(offu[:], offf[:])

                # ---- gather the selected chunks, rescan for top-8 each ----
                g24 = gbig.tile([128, NSEL * CW], F16, tag="g24")
                nc.gpsimd.indirect_dma_start(
                    out=g24[:].rearrange("p (k w) -> p k w", w=CW),
                    out_offset=None, in_=affsD[:],
                    in_offset=bass.IndirectOffsetOnAxis(ap=offu[:], axis=0))
                # multiplicative positional eps: v' = v*(1 + pos*2^-19).
                # v is on the f16 grid, so round-to-f16 recovers v and the
                # ratio recovers pos -- no max_index pass needed.
                g24f = gbig.tile([128, NSEL * CW], F32, tag="g24f")
                nc.vector.tensor_tensor(
                    out=g24f[:], in0=g24[:], in1=ep1536[:],
                    op=mybir.AluOpType.mult)
                cv8 = sel.tile([128, NCAND], F32, tag="cv8")
                for j in range(NPR):
                    nc.vector.max(
                        out=cv8[:, j * 8:(j + 1) * 8],
                        in_=g24f[:, j * 2 * CW:(j + 1) * 2 * CW])
                vq16 = sel.tile([128, NCAND], F16, tag="vq16")
                nc.vector.tensor_copy(vq16[:], cv8[:])
                # denominator guard: +-1e-30 by sign so v=0 can't divide by 0
                sg = sel.tile([128, NCAND], F32, tag="sg")
                nc.vector.tensor_scalar(
                    sg[:], vq16[:], 0.0, None, op0=mybir.AluOpType.is_ge)
                nc.vector.tensor_scalar(
                    sg[:], sg[:], 2e-30, -1e-30,
                    op0=mybir.AluOpType.mult, op1=mybir.AluOpType.add)
                vq2 = sel.tile([128, NCAND], F32, tag="vq2")
                nc.vector.tensor_tensor(
                    out=vq2[:], in0=vq16[:], in1=sg[:],
                    op=mybir.AluOpType.add)
                pf = sel.tile([128, NCAND], F32, tag="pf")
                nc.vector.tensor_tensor(
                    out=pf[:], in0=cv8[:], in1=vq2[:],
                    op=mybir.AluOpType.divide)
                nc.vector.tensor_scalar(
                    pf[:], pf[:], float(2 ** 21), -float(2 ** 21) + 0.5,
                    op0=mybir.AluOpType.mult, op1=mybir.AluOpType.add)
                nc.vector.tensor_scalar(
                    pf[:], pf[:], 0.0, None, op0=mybir.AluOpType.max)
                nc.vector.tensor_scalar(
                    pf[:], pf[:], 127.9, None, op0=mybir.AluOpType.min)
                # pair decode: el = cid[2j]*64 + pf, or the odd chunk's base
                # when pf falls in the second half of the gathered pair
                cid3 = cidf[:].rearrange("p (j two) -> p j two", two=2)
                cdm = sel.tile([128, NPR], F32, tag="cdm")
                nc.vector.tensor_tensor(
                    out=cdm[:], in0=cid3[:, :, 1], in1=cid3[:, :, 0],
                    op=mybir.AluOpType.subtract)
                nc.vector.tensor_scalar(
                    cdm[:], cdm[:], float(CW), -float(CW),
                    op0=mybir.AluOpType.mult, op1=mybir.AluOpType.add)
                ge = sel.tile([128, NCAND], F32, tag="ge")
                nc.vector.tensor_scalar(
                    ge[:], pf[:], float(CW), None, op0=mybir.AluOpType.is_ge)
                nc.vector.tensor_tensor(
                    out=ge[:].rearrange("p (j r) -> p j r", r=8),
                    in0=ge[:].rearrange("p (j r) -> p j r", r=8),
                    in1=cdm[:].rearrange("p (j u) -> p j u", u=1)
                    .broadcast_to([128, NPR, 8]),
                    op=mybir.AluOpType.mult)
                elf = sel.tile([128, NCAND], F32, tag="elf")
                nc.vector.scalar_tensor_tensor(
                    out=elf[:].rearrange("p (j r) -> p j r", r=8),
                    in0=cid3[:, :, 0:1].broadcast_to([128, NPR, 8]),
                    scalar=float(CW),
                    in1=pf[:].rearrange("p (j r) -> p j r", r=8),
                    op0=mybir.AluOpType.mult, op1=mybir.AluOpType.add)
                nc.vector.tensor_tensor(
                    out=elf[:], in0=elf[:], in1=ge[:],
                    op=mybir.AluOpType.add)
                nc.sync.dma_start(
                    out=elD[:].rearrange("(p u) one -> p (u one)", p=128),
                    in_=elf[:])

                # ---- merge: exact top-20 of the 192 candidates ----
                cvf = sel.tile([128, NCAND], F32, tag="cvf")
                nc.vector.tensor_tensor(
                    out=cvf[:], in0=cv8[:], in1=ep192[:],
                    op=mybir.AluOpType.add)
                gvals = sel.tile([128, 24], F32, tag="gvals")
                gpos = sel.tile([128, 24], U32, tag="gpos")
                for r in range(3):
                    g8 = gvals[:, r * 8:(r + 1) * 8]
                    nc.vector.max(out=g8, in_=cvf[:])
                    nc.vector.max_index(
                        out=gpos[:, r * 8:(r + 1) * 8], in_max=g8,
                        in_values=cvf[:])
                    if r < 2:
                        nc.vector.match_replace(
                            out=cvf[:], in_to_replace=g8, in_values=cvf[:],
                            imm_value=NEG)
                # ---- exact fp32 rescore of the 24 candidates ----
                gposf = sel.tile([128, 24], F32, tag="gposf")
                nc.vector.tensor_copy(gposf[:], gpos[:])
                off2 = sel.tile([128, 24], F32, tag="off2")
                nc.vector.tensor_scalar(
                    off2[:], gposf[:], pr192[:], None, op0=mybir.AluOpType.add)
                offu2 = sel.tile([128, 24], U32, tag="offu2")
                nc.vector.tensor_copy(offu2[:], off2[:])
                el24 = sel.tile([128, 24], F32, tag="el24")
                nc.gpsimd.indirect_dma_start(
                    out=el24[:], out_offset=None, in_=elD[:],
                    in_offset=bass.IndirectOffsetOnAxis(ap=offu2[:], axis=0))
                el24u = sel.tile([128, 24], U32, tag="el24u")
                nc.vector.tensor_copy(el24u[:], el24[:])
                gmk = gbig.tile([128, 24 * (CK + 1)], F32, tag="gmk")
                nc.gpsimd.indirect_dma_start(
                    out=gmk[:].rearrange("p (k c) -> p k c", c=CK + 1),
                    out_offset=None, in_=mkT32[:],
                    in_offset=bass.IndirectOffsetOnAxis(ap=el24u[:], axis=0))
                # V rows for all 24 candidates, issued early so the gather
                # overlaps the rescore; non-top-20 slots get zero weight
                vTg = gbig.tile([128, 24 * 2 * CV], F16, tag="vTg")
                for hb in range(2):
                    nc.gpsimd.indirect_dma_start(
                        out=vTg[:, hb * 12 * 2 * CV:(hb + 1) * 12 * 2 * CV]
                        .rearrange("p (k c) -> p k c", c=2 * CV),
                        out_offset=None, in_=vTb[:],
                        in_offset=bass.IndirectOffsetOnAxis(
                            ap=el24u[:, hb * 12:(hb + 1) * 12], axis=0))
                nc.vector.tensor_tensor(
                    out=gmk[:].rearrange("p (k c) -> p k c", c=CK + 1),
                    in0=gmk[:].rearrange("p (k c) -> p k c", c=CK + 1),
                    in1=qrt[:].rearrange("p (u c) -> p u c", u=1)
                    .broadcast_to([128, 24, CK + 1]),
                    op=mybir.AluOpType.mult)
                av24 = sel.tile([128, 24], F32, tag="av24")
                nc.vector.tensor_reduce(
                    out=av24[:],
                    in_=gmk[:].rearrange("p (k c) -> p k c", c=CK + 1),
                    axis=mybir.AxisListType.X, op=mybir.AluOpType.add)
                nc.vector.tensor_tensor(
                    out=av24[:], in0=av24[:], in1=ep192[:, :24],
                    op=mybir.AluOpType.add)
                # ranked values (top-20 threshold), no positions needed
                av24c = sel.tile([128, 24], F32, tag="av24c")
                nc.vector.tensor_copy(av24c[:], av24[:])
                wvals = sel.tile([128, 24], F32, tag="wvals")
                for r in range(3):
                    w8 = wvals[:, r * 8:(r + 1) * 8]
                    nc.vector.max(out=w8, in_=av24c[:])
                    if r < 2:
                        nc.vector.match_replace(
                            out=av24c[:], in_to_replace=w8, in_values=av24c[:],
                            imm_value=NEG)

                # ---- masked softmax over all 24 slots (ranks >= 20 -> 0) ---
                mask = sel.tile([128, 24], F32, tag="mask")
                nc.vector.tensor_scalar(
                    mask[:], av24[:], wvals[:, 19:20], None,
                    op0=mybir.AluOpType.is_ge)
                negm40 = sel.tile([128, 1], F32, tag="negm40")
                nc.vector.tensor_scalar(
                    negm40[:], wvals[:, 0:1], -1.0, -40.0,
                    op0=mybir.AluOpType.mult, op1=mybir.AluOpType.add)
                # masked slots sit 40 below the kept ones -> exp ~ 0
                dms = sel.tile([128, 24], F32, tag="dms")
                nc.vector.scalar_tensor_tensor(
                    out=dms[:], in0=mask[:], scalar=40.0, in1=av24[:],
                    op0=mybir.AluOpType.mult, op1=mybir.AluOpType.add)
                wexp = sel.tile([128, 24], F32, tag="wexp")
                ssum = sel.tile([128, 1], F32, tag="ssum")
                nc.scalar.activation(
                    out=wexp[:], in_=dms[:],
                    func=mybir.ActivationFunctionType.Exp,
                    bias=negm40[:], scale=1.0, accum_out=ssum[:])
                rs = sel.tile([128, 1], F32, tag="rs")
                nc.vector.reciprocal(rs[:], ssum[:])
                wgt = sel.tile([128, 24], F32, tag="wgt")
                nc.vector.tensor_scalar(
                    wgt[:], wexp[:], rs[:], None, op0=mybir.AluOpType.mult)

                # ---- weighted readout over the 24 gathered V rows ----
                # slots 12..23 are scaled on the Activation engine and
                # pair-summed; DVE accumulates slots 0..11 directly
                nact = 12 if t < NQT - 1 else 6
                terms = gbig.tile([128, 12 * 2 * CV], F16, tag="terms")
                for j in range(nact):
                    k = 24 - nact + j
                    nc.scalar.activation(
                        out=terms[:, j * 2 * CV:(j + 1) * 2 * CV],
                        in_=vTg[:, k * 2 * CV:(k + 1) * 2 * CV],
                        func=mybir.ActivationFunctionType.Copy,
                        scale=wgt[:, k:k + 1])
                acc = gat.tile([128, 2 * CV], F32, tag="acc")
                nc.vector.tensor_scalar(
                    acc[:], vTg[:, 0:2 * CV], wgt[:, 0:1], None,
                    op0=mybir.AluOpType.mult)
                for k in range(1, 24 - nact):
                    nc.vector.scalar_tensor_tensor(
                        out=acc[:], in0=vTg[:, k * 2 * CV:(k + 1) * 2 * CV],
                        scalar=wgt[:, k:k + 1], in1=acc[:],
                        op0=mybir.AluOpType.mult, op1=mybir.AluOpType.add)
                # pairwise-sum the Act-scaled terms, then fold into acc
                n = nact
                while n > 1:
                    h = n // 2
                    nc.vector.tensor_tensor(
                        out=terms[:, 0:h * 2 * CV],
                        in0=terms[:, 0:h * 2 * CV],
                        in1=terms[:, h * 2 * CV:2 * h * 2 * CV],
                        op=mybir.AluOpType.add)
                    if n % 2:
                        nc.vector.tensor_tensor(
                            out=acc[:], in0=terms[:, (n - 1) * 2 * CV:n * 2 * CV],
                            in1=acc[:], op=mybir.AluOpType.add)
                    n = h
                nc.vector.tensor_tensor(
                    out=acc[:], in0=terms[:, 0:2 * CV], in1=acc[:],
                    op=mybir.AluOpType.add)
                nc.sync.dma_start(
                    out=out[t * 128:(t + 1) * 128, :], in_=acc[:])
    nc.finalize()
    _prog_cache["p"] = nc
    return nc


def _host_inputs(qk, mem_k, mem_v1, mem_v2, top_k=TOPK):
    qk = np.asarray(qk, dtype=np.float32)
    mem_k = np.asarray(mem_k, dtype=np.float32)
    mem_v1 = np.asarray(mem_v1, dtype=np.float32)
    mem_v2 = np.asarray(mem_v2, dtype=np.float32)

    q2 = qk.reshape(CK, Q)
    a = np.sum(mem_k[0] * mem_k[0], axis=0, dtype=np.float32)      # [NE]
    na = -0.125 * a
    nh = na.astype(np.float16).astype(np.float32)
    nl = (na - nh).astype(np.float16)
    mkB = np.concatenate(
        [mem_k[0].astype(np.float16), nh.astype(np.float16)[None, :],
         nl[None, :]], axis=0)                                      # [66, NE]
    vTb = np.concatenate(
        [mem_v1[0].T, mem_v2[0].T], axis=1).astype(np.float16)      # [NE, 512]
    mkT32 = np.ascontiguousarray(np.concatenate(
        [mem_k[0].T, na[:, None]], axis=1, dtype=np.float32))      # [NE, 65]
    prow512 = (np.arange(128, dtype=np.float32) * NCH).reshape(128, 1)
    prow192 = (np.arange(128, dtype=np.float32) * NCAND).reshape(128, 1)
    eps512 = np.broadcast_to(
        np.arange(NCH, dtype=np.float32) * EPS, (128, NCH)).copy()
    eps192 = np.broadcast_to(
        np.arange(NCAND, dtype=np.float32) * EPS, (128, NCAND)).copy()
    eps1536 = np.broadcast_to(
        1.0 + (np.arange(NSEL * CW, dtype=np.float32) % (2 * CW)) * 2.0 ** -21,
        (128, NSEL * CW)).astype(np.float32).copy()

    in_maps = []
    for c in range(NC):
        sl = slice(c * Q_LOC, (c + 1) * Q_LOC)
        qs = 0.25 * q2[:, sl]
        qTb = np.concatenate(
            [qs.astype(np.float16), np.ones((2, Q_LOC), np.float16)],
            axis=0)                                                 # [66, 512]
        qrI = np.ascontiguousarray(np.concatenate(
            [qs.T, np.ones((Q_LOC, 1), np.float32)],
            axis=1, dtype=np.float32))                              # [512, 65]
        in_maps.append({
            "qTb": qTb, "mkB": mkB, "vTb": vTb,
            "mkT32": mkT32, "qrI": qrI,
            "prow512": prow512, "prow192": prow192,
            "eps512": eps512, "eps192": eps192, "eps1536": eps1536,
        })
    return in_maps


def _assemble_output(outs):
    full = np.concatenate(outs, axis=0)
    return np.ascontiguousarray(full.T).reshape(1, 2 * CV, H, W)


def kernel(qk, mem_k, mem_v1, mem_v2, top_k):
    assert int(top_k) == TOPK
    in_maps = _host_inputs(qk, mem_k, mem_v1, mem_v2)
    nc = _build_program()
    res = None
    for attempt in range(3):
        try:
            res = run_bass_kernel_spmd(nc, in_maps, core_ids=list(range(NC)))
            break
        except Exception:
            # transient device-unrecoverable states clear on the next attempt
            if attempt == 2:
                raise
            time.sleep(2.0)
    return _assemble_output([res.results[c]["out"] for c in range(NC)])
